# revision 12
# baseline (speedup 1.0000x reference)
"""Trainium2 Bass kernel: Conv2d(1->64,3x3) + 3-layer GRU over T=256.

Weight-stationary formulation (v2):
  - Conv folded into layer-0 input weights host-side (gi0[t] = W_eff @ x3[t]).
  - 8 cores = 2 batch halves (32 each) x 4 time chunks with WARM-step warmup
    (GRU state contracts ~0.77/step, so warm chunks converge).
  - All GEMMs keep the WEIGHTS as the stationary operand and h as the moving
    operand, producing gates in [gate, batch] layout: each matmul streams only
    BL=32 columns, every elementwise op runs 128 partitions wide, and h never
    needs transposing (h' is produced directly in the layout the next step's
    matmuls consume).
  - Per (layer, step): one PSUM bank [128, 512] holds 16 blocks of 32 cols:
    blocks 0..7 = r|z pre-acts (gi+gh+bias), 8..11 = gi_n, 12..15 = gh_n.
    Biases enter via a single K=16 selector matmul that writes the whole bank
    (start=True), then gi/gh chunk matmuls accumulate block-wise.
  - Eltwise: sigmoid (ACT, psum->sbuf bf16), n = tanh(gi_n + r*gh_n) (DVE mul/
    add + ACT tanh), h' = n + z*(h-n) (DVE bf16). h state lives in bf16.
  - Wavefront: span s runs (l=0,t=s), (l=1,t=s-1), (l=2,t=s-2); all cross-
    group deps come from the previous span, so groups in a span are
    independent and the eltwise chain of one group hides under the PE stream
    of the others.
  - Single-sync-wait discipline (walrus limit): cheap absorber instructions
    (PE ldweights / 1-elem DVE copies / Pool memsets) carry all but one of
    each real instruction's cross-engine waits; Tile's wait assignment then
    elides the dominated ones.
  - Weights/bias/x3 stream in via SP-issued (HWDGE) DMAs; per-step output
    DMAs via gpsimd (SWDGE) read the layer-2 h tile directly.
"""

import sys

for _p in ("/opt/trn_rl_repo",):
    if _p not in sys.path:
        sys.path.insert(0, _p)

import numpy as np
import ml_dtypes

import concourse.bass as bass
import concourse.mybir as mybir
import concourse.tile as tile
from concourse.bass import _add_dep_helper
from concourse.bass_utils import run_bass_kernel_spmd

BF16 = mybir.dt.bfloat16
F32 = mybir.dt.float32
AF = mybir.ActivationFunctionType

B, NB, T, F, H = 64, 64, 256, 64, 512
G3 = 3 * H  # 1536
KX = 3 * NB  # 192 folded-conv contraction
BATCH_WAYS = 2
N_CHUNKS = 4
WARM = 32
S = (T + (N_CHUNKS - 1) * WARM) // N_CHUNKS  # 88 steps per core
BL = B // BATCH_WAYS  # 32 batch rows per core
NT = 12  # gate tiles of 128 (1536/128)

_NC_CACHE: dict = {}
_DBG_LABELS: dict = {}  # inst name -> semantic label (debug aid)


def _lbl(h, label):
    _DBG_LABELS[h.ins.name] = label
    return h


def _build_nc(s_steps: int = S, bl: int = BL):
    nc = bass.Bass()

    wg0_ext = nc.declare_dram_parameter("wg0", [2, 128, G3], BF16, isOutput=False)
    wg12_ext = nc.declare_dram_parameter("wg12", [8, 128, G3], BF16, isOutput=False)
    wh_ext = nc.declare_dram_parameter("wh", [12, 128, G3], BF16, isOutput=False)
    bmat_ext = nc.declare_dram_parameter("bmat", [16, 3 * 128], BF16, isOutput=False)
    sel_ext = nc.declare_dram_parameter("sel", [16, 512], BF16, isOutput=False)
    x3t_ext = nc.declare_dram_parameter("x3t", [s_steps, 128, 64], BF16, isOutput=False)
    h0t_ext = nc.declare_dram_parameter("h0t", [3, 128, 4 * bl], BF16, isOutput=False)
    out_ext = nc.declare_dram_parameter("out", [s_steps, 128, 4 * bl], BF16,
                                        isOutput=True)

    from contextlib import ExitStack

    with tile.TileContext(nc) as tc, ExitStack() as ctx:
        wpool = ctx.enter_context(tc.tile_pool(name="static", bufs=1))
        hT_pools = [
            ctx.enter_context(tc.tile_pool(name=f"hT{l}", bufs=(8 if l == 2 else 3)))
            for l in range(3)
        ]
        rz_pools = [ctx.enter_context(tc.tile_pool(name=f"rz{l}", bufs=3))
                    for l in range(3)]
        nmul_pools = [ctx.enter_context(tc.tile_pool(name=f"nmul{l}", bufs=3))
                      for l in range(3)]
        npre_pools = [ctx.enter_context(tc.tile_pool(name=f"npre{l}", bufs=3))
                      for l in range(3)]
        nt_pools = [ctx.enter_context(tc.tile_pool(name=f"nt{l}", bufs=3))
                    for l in range(3)]
        d_pools = [ctx.enter_context(tc.tile_pool(name=f"d{l}", bufs=3))
                   for l in range(3)]
        zd_pools = [ctx.enter_context(tc.tile_pool(name=f"zd{l}", bufs=3))
                    for l in range(3)]
        ps_pools = [
            ctx.enter_context(tc.tile_pool(name=f"ps{l}", bufs=2, space="PSUM"))
            for l in range(3)
        ]

        # ---- static SBUF slabs ------------------------------------------
        wg0_sb = wpool.tile([128, 2 * G3], BF16, tag="wg0")
        wg12_sb = wpool.tile([128, 8 * G3], BF16, tag="wg12")
        wh_sb = wpool.tile([128, 12 * G3], BF16, tag="wh")
        bmat_sb = wpool.tile([128, 3 * 128], BF16, tag="bmat")
        sel_sb = wpool.tile([128, 512], BF16, tag="sel")
        x3_sb = wpool.tile([128, s_steps * 64], BF16, tag="x3")
        h0_sb = wpool.tile([128, 3 * 4 * bl], BF16, tag="h0")
        dummy = wpool.tile([1, 2048], F32, tag="dummy")
        act_dummy = wpool.tile([1, 2048], F32, tag="actdummy")
        dummy_ctr = [0]
        act_ctr = [0]

        # ---- preamble DMAs (SP / HWDGE) ---------------------------------
        sp_dmas = []

        def _sdma(dst, src):
            d = nc.sync.dma_start(dst, src)
            sp_dmas.append(d)
            return d

        wg_last = [None, None, None]  # last DMA per layer's gi slab
        wh_last = [None, None, None]
        x3_dma = [None] * s_steps

        def _x3range(lo, hi):
            for i in range(lo, min(hi, s_steps)):
                x3_dma[i] = _sdma(x3_sb[:, i * 64:(i + 1) * 64], x3t_ext[i])

        for c in range(2):
            wg_last[0] = _sdma(wg0_sb[:, c * G3:(c + 1) * G3], wg0_ext[c])
        for c in range(4):
            wh_last[0] = _sdma(wh_sb[:, c * G3:(c + 1) * G3], wh_ext[c])
        bmat_dma = _sdma(bmat_sb[0:16, :], bmat_ext[:, :])
        sel_dma = _sdma(sel_sb[0:16, :], sel_ext[:, :])
        h0_dma = [None] * 3
        for l in range(3):
            h0_dma[l] = _sdma(h0_sb[:, l * 4 * bl:(l + 1) * 4 * bl], h0t_ext[l])
        _x3range(0, 4)
        for c in range(4):
            wg_last[1] = _sdma(wg12_sb[:, c * G3:(c + 1) * G3], wg12_ext[c])
        for c in range(4):
            wh_last[1] = _sdma(wh_sb[:, (4 + c) * G3:(5 + c) * G3], wh_ext[4 + c])
        _x3range(4, 12)
        for c in range(4):
            wg_last[2] = _sdma(wg12_sb[:, (4 + c) * G3:(5 + c) * G3], wg12_ext[4 + c])
        for c in range(4):
            wh_last[2] = _sdma(wh_sb[:, (8 + c) * G3:(9 + c) * G3], wh_ext[8 + c])
        _x3range(12, s_steps)

        # PE absorbers so the first bias matmul never carries DMA waits
        for dep in (bmat_dma, sel_dma):
            a = nc.tensor.ldweights(bmat_sb[0:1, 0:1])
            _add_dep_helper(a.ins, dep.ins, sync=True, reason="preamble prime")

        # ---- bookkeeping -------------------------------------------------
        hT = [dict() for _ in range(3)]     # (l, i) -> h tile [128, 4*bl]
        hprod = [dict() for _ in range(3)]  # (l, i) -> producing instruction
        sig_hist = [dict() for _ in range(3)]
        tanh_hist = [dict() for _ in range(3)]
        add1_hist = [dict() for _ in range(3)]
        out_dmas = []
        last_eng = {}

        for l in range(3):
            hT[l][-1] = h0_sb[:, l * 4 * bl:(l + 1) * 4 * bl]

        def ldw_abs(dep, reason):
            a = nc.tensor.ldweights(bmat_sb[0:1, 0:1])
            _add_dep_helper(a.ins, dep.ins, sync=True, reason=reason)
            return a

        def dve_abs_read(src_ap):
            c = dummy_ctr[0] % 2048
            dummy_ctr[0] += 1
            return nc.vector.tensor_copy(dummy[0:1, c:c + 1], src_ap)

        act_zero = nc.const_aps.scalar_like(0.0, act_dummy[0:1, 0:1])

        def act_abs(dep, reason):
            c = act_ctr[0] % 2048
            act_ctr[0] += 1
            a = nc.scalar.activation(act_dummy[0:1, c:c + 1], act_zero, AF.Copy)
            _add_dep_helper(a.ins, dep.ins, sync=True, reason=reason)
            return a

        def emit_group(l, i):
            # --- wait absorbers (keep every real instruction at <=1 wait) --
            grp_abs = []
            if i == 0:
                grp_abs.append(ldw_abs(wg_last[l], f"wg{l} slab ready"))
                grp_abs.append(ldw_abs(wh_last[l], f"wh{l} slab ready"))
                grp_abs.append(ldw_abs(h0_dma[l], "h0 slab ready"))
            else:
                grp_abs.append(
                    ldw_abs(hprod[l][i - 1], "h[l][i-1] ready (covers h[l-1][i])"))
                if l >= 1:
                    grp_abs.append(
                        ldw_abs(hprod[l - 1][i], "h[l-1][i] ready"))
            if i >= 2:
                grp_abs.append(ldw_abs(sig_hist[l][i - 2], "psum WAR vs old sig"))
                grp_abs.append(ldw_abs(add1_hist[l][i - 2], "psum WAR vs old add1"))

            ps = ps_pools[l].tile([128, 512], F32, tag=f"ps{l}")

            # --- matmuls --------------------------------------------------
            # bias: psum[p, c] = bmat[c//32, p] for the whole bank
            mm_bias = _lbl(nc.tensor.matmul(
                ps[:, :], bmat_sb[0:16, l * 128:(l + 1) * 128],
                sel_sb[0:16, :], start=True, stop=False,
                skip_group_check=True), f"mm_bias l{l} i{i}")
            for a in grp_abs:
                _add_dep_helper(mm_bias.ins, a.ins, sync=False,
                                reason="group absorbers precede first matmul")

            if l == 0:
                gi_src = [(x3_sb[0:128, i * 64:i * 64 + 32], 0),
                          (x3_sb[0:64, i * 64 + 32:i * 64 + 64], 1)]

                def gi_w(c, j):
                    return wg0_sb[0:(128 if c == 0 else 64),
                                  c * G3 + j * 128:c * G3 + (j + 1) * 128]
            else:
                hsrc = hT[l - 1][i]
                gi_src = [(hsrc[:, c * bl:(c + 1) * bl], c) for c in range(4)]

                def gi_w(c, j):
                    base = ((l - 1) * 4 + c) * G3
                    return wg12_sb[:, base + j * 128:base + (j + 1) * 128]

            ghsrc = hT[l][i - 1]
            gh_src = [(ghsrc[:, c * bl:(c + 1) * bl], c) for c in range(4)]

            def gh_w(c, j):
                base = (l * 4 + c) * G3
                return wh_sb[:, base + j * 128:base + (j + 1) * 128]

            mms = []
            # n-path blocks first (8..11 gi_n, 12..15 gh_n), then r|z (0..7)
            for j in range(8, 12):
                for rhs, c in gi_src:
                    mms.append((gi_w(c, j), rhs, j))
            for j in range(8, 12):
                for rhs, c in gh_src:
                    mms.append((gh_w(c, j), rhs, j + 4))
            for j in range(0, 8):
                for rhs, c in gi_src:
                    mms.append((gi_w(c, j), rhs, j))
                for rhs, c in gh_src:
                    mms.append((gh_w(c, j), rhs, j))
            n = len(mms)
            prev_mm = mm_bias
            for idx, (w, rhs, blk) in enumerate(mms):
                kk = w.shape[0]
                prev_mm = _lbl(nc.tensor.matmul(
                    ps[:, blk * bl:(blk + 1) * bl], w, rhs[0:kk, :],
                    start=False, stop=(idx == n - 1), skip_group_check=True),
                    f"mm l{l} i{i} #{idx} blk{blk}")
            last_eng['PE'] = prev_mm

            # --- eltwise --------------------------------------------------
            # r|z sigmoid -> sbuf bf16
            rz = rz_pools[l].tile([128, 2 * 4 * bl], BF16, tag=f"rz{l}")
            if i >= 3:
                sa = act_abs(sig_hist[l][i - 3], "rz WAW completion")
            sig = _lbl(nc.scalar.activation(rz[:, :], ps[:, 0:8 * bl], AF.Sigmoid),
                       f"sig l{l} i{i}")
            if i >= 3:
                _add_dep_helper(sig.ins, sa.ins, sync=False,
                                reason="WAW absorber precedes sigmoid")
            sig_hist[l][i] = sig

            # absorb the sigmoid tick into the DVE clock
            dabs = _lbl(dve_abs_read(rz[0:1, 0:1]), f"dve_abs_sig l{l} i{i}")
            pre_mul = [dabs]
            if i == 0:
                pre_mul.append(_lbl(
                    dve_abs_read(h0_sb[0:1, l * 4 * bl:l * 4 * bl + 1]),
                    f"dve_abs_h0 l{l}"))
            # n = tanh(gi_n + r * gh_n)
            nm = nmul_pools[l].tile([128, 4 * bl], F32, tag=f"nmul{l}")
            mul1 = _lbl(nc.vector.tensor_mul(nm[:, :], rz[:, 0:4 * bl],
                                             ps[:, 12 * bl:16 * bl]),
                        f"mul1 l{l} i{i}")
            for a in pre_mul:
                _add_dep_helper(mul1.ins, a.ins, sync=False,
                                reason="absorbers precede n-path mult")
            npre = npre_pools[l].tile([128, 4 * bl], F32, tag=f"npre{l}")
            add1 = _lbl(nc.vector.tensor_add(npre[:, :], nm[:, :],
                                             ps[:, 8 * bl:12 * bl]),
                        f"add1 l{l} i{i}")
            add1_hist[l][i] = add1
            ntl = nt_pools[l].tile([128, 4 * bl], BF16, tag=f"nt{l}")
            if i >= 3:
                ta = act_abs(tanh_hist[l][i - 3], "nt WAW completion")
            tanh = _lbl(nc.scalar.activation(ntl[:, :], npre[:, :], AF.Tanh),
                        f"tanh l{l} i{i}")
            if i >= 3:
                _add_dep_helper(tanh.ins, ta.ins, sync=False,
                                reason="WAW absorber precedes tanh")
            tanh_hist[l][i] = tanh
            last_eng['ACT'] = tanh

            # h' = n + z*(h - n)   (all bf16, SBUF)
            dt_ = d_pools[l].tile([128, 4 * bl], BF16, tag=f"d{l}")
            sub = _lbl(nc.vector.tensor_sub(dt_[:, :], ghsrc[:, :], ntl[:, :]),
                       f"sub l{l} i{i}")
            zd = zd_pools[l].tile([128, 4 * bl], BF16, tag=f"zd{l}")
            zdm = _lbl(nc.vector.tensor_mul(zd[:, :], rz[:, 4 * bl:8 * bl],
                                            dt_[:, :]), f"zd l{l} i{i}")
            pre_hp = []
            if l == 2 and i >= 8:
                # hT2 slot recycle: absorb the old out-DMA tick into DVE
                a = _lbl(dve_abs_read(nc.const_aps.scalar_like(0.0, dummy[0:1, 0:1])), f"dve_abs_odma i{i}")
                _add_dep_helper(a.ins, out_dmas[i - 8].ins, sync=True,
                                reason="hT2 WAR vs old out DMA")
                pre_hp.append(a)
            hnew = hT_pools[l].tile([128, 4 * bl], BF16, tag=f"hT{l}")
            hp = _lbl(nc.vector.tensor_add(hnew[:, :], zd[:, :], ntl[:, :]),
                      f"hp l{l} i{i}")
            for a in pre_hp:
                _add_dep_helper(hp.ins, a.ins, sync=False,
                                reason="out-DMA absorber precedes h'")
            last_eng['DVE'] = hp
            hT[l][i] = hnew
            hprod[l][i] = hp
            if i - 2 in hT[l] and l != 2:
                del hT[l][i - 2]

            if l == 2:
                prev_dma = None
                if len(out_dmas) >= 8:
                    c = dummy_ctr[0] % 2048
                    dummy_ctr[0] += 1
                    m = _lbl(nc.gpsimd.memset(dummy[0:1, c:c + 1], 0.0),
                             f"pool_abs i{i}")
                    _add_dep_helper(m.ins, out_dmas[-8].ins, sync=True,
                                    reason="SWDGE ring slot")
                    last_eng['POOL'] = m
                    prev_dma = m
                dma = _lbl(nc.gpsimd.dma_start(out_ext[i], hnew[:, :]),
                           f"out_dma i{i}")
                if prev_dma is not None:
                    _add_dep_helper(dma.ins, prev_dma.ins, sync=False,
                                    reason="ring absorber precedes out DMA")
                out_dmas.append(dma)
                if i - 9 in hT[2]:
                    del hT[2][i - 9]

        for s in range(s_steps + 2):
            for l in range(3):
                i = s - l
                if 0 <= i < s_steps:
                    emit_group(l, i)

        # ---- tail pre-drains (final Drain must see <=1 new wait) ---------
        tail = list(last_eng.values()) + out_dmas[-8:] + sp_dmas[-8:]
        for dep in tail:
            dr = nc.sync.drain(fusable=False)
            _add_dep_helper(dr.ins, dep.ins, sync=True, reason="tail pre-drain")

    return nc


# ---------------------------------------------------------------------------
# Host-side input preparation


def _fold_conv(conv_w, conv_b, w_ih0, b_ih0):
    """Fold conv into layer0 input weights: gi0[t] = W_eff @ x3[t] + b_eff."""
    RNN_IN = F * (NB - 2)
    C = np.zeros((RNN_IN, KX), np.float64)
    for f in range(F):
        for di in range(3):
            for dt in range(3):
                w = float(conv_w[f, 0, di, dt])
                for i in range(NB - 2):
                    C[f * (NB - 2) + i, dt * NB + (i + di)] += w
    W_eff = w_ih0.astype(np.float64) @ C  # [1536, 192]
    bc = np.repeat(conv_b.astype(np.float64), NB - 2)
    b_eff = b_ih0.astype(np.float64) + w_ih0.astype(np.float64) @ bc
    return W_eff.astype(np.float32), b_eff.astype(np.float32)


def _bf16(a):
    return np.ascontiguousarray(a.astype(ml_dtypes.bfloat16))


def _prep_core_inputs(inputs, s_steps=S, warm=WARM):
    x = np.asarray(inputs["x"], np.float32)
    W_eff, b_eff = _fold_conv(np.asarray(inputs["conv_w"], np.float32),
                              np.asarray(inputs["conv_b"], np.float32),
                              np.asarray(inputs["w_ih0"], np.float32),
                              np.asarray(inputs["b_ih0"], np.float32))

    wg0 = np.zeros((2, 128, G3), np.float32)
    WeT = W_eff.T  # [192, 1536]
    wg0[0] = WeT[0:128]
    wg0[1, 0:64] = WeT[128:192]
    wg12 = np.zeros((8, 128, G3), np.float32)
    for l in (1, 2):
        wiT = np.asarray(inputs[f"w_ih{l}"], np.float32).T
        for c in range(4):
            wg12[(l - 1) * 4 + c] = wiT[c * 128:(c + 1) * 128]
    wh = np.zeros((12, 128, G3), np.float32)
    for l in range(3):
        whT = np.asarray(inputs[f"w_hh{l}"], np.float32).T
        for c in range(4):
            wh[l * 4 + c] = whT[c * 128:(c + 1) * 128]

    bmat = np.zeros((16, 3 * 128), np.float32)
    for l in range(3):
        b_h = np.asarray(inputs[f"b_hh{l}"], np.float32)
        b_i = b_eff if l == 0 else np.asarray(inputs[f"b_ih{l}"], np.float32)
        col = slice(l * 128, (l + 1) * 128)
        both = b_i + b_h
        for k in range(8):
            bmat[k, col] = both[k * 128:(k + 1) * 128]
        for k in range(4):
            bmat[8 + k, col] = b_i[1024 + k * 128:1024 + (k + 1) * 128]
            bmat[12 + k, col] = b_h[1024 + k * 128:1024 + (k + 1) * 128]
    sel = np.zeros((16, 512), np.float32)
    for k in range(16):
        sel[k, k * 32:(k + 1) * 32] = 1.0

    wg0_b, wg12_b, wh_b = _bf16(wg0), _bf16(wg12), _bf16(wh)
    bmat_b, sel_b = _bf16(bmat), _bf16(sel)

    x2p = np.pad(x[:, 0], ((0, 0), (0, 0), (1, 1)))  # [B, NB, T+2]
    hs = [np.asarray(inputs[f"h{l + 1}"], np.float32) for l in range(3)]

    in_maps = []
    chunk_starts = [0] + [s_steps + (j - 1) * (s_steps - warm) - warm
                          for j in range(1, N_CHUNKS)]
    for bh in range(BATCH_WAYS):
        bsl = slice(bh * BL, (bh + 1) * BL)
        for j in range(N_CHUNKS):
            t0 = chunk_starts[j]
            seg = x2p[bsl, :, t0:t0 + s_steps + 2]  # [BL, 64, S+2]
            A = np.stack([seg[:, :, dt:dt + s_steps] for dt in range(3)], axis=0)
            w3T = A.transpose(0, 2, 1, 3).reshape(KX, BL, s_steps)  # [k, b, i]
            x3t = np.zeros((s_steps, 128, 64), np.float32)
            x3t[:, :, 0:32] = w3T[0:128].transpose(2, 0, 1)
            x3t[:, 0:64, 32:64] = w3T[128:192].transpose(2, 0, 1)
            h0t = np.zeros((3, 128, 4 * BL), np.float32)
            if j == 0:
                for l in range(3):
                    hT0 = hs[l][bsl].T  # [H, BL]
                    for c in range(4):
                        h0t[l, :, c * BL:(c + 1) * BL] = hT0[c * 128:(c + 1) * 128]
            in_maps.append({
                "wg0": wg0_b, "wg12": wg12_b, "wh": wh_b,
                "bmat": bmat_b, "sel": sel_b,
                "x3t": _bf16(x3t), "h0t": _bf16(h0t),
            })
    return in_maps, chunk_starts


def kernel(**inputs) -> np.ndarray:
    if "nc" not in _NC_CACHE:
        _NC_CACHE["nc"] = _build_nc()
    nc = _NC_CACHE["nc"]
    in_maps, chunk_starts = _prep_core_inputs(inputs)
    res = run_bass_kernel_spmd(nc, in_maps, list(range(8)))
    _NC_CACHE["last_result"] = res
    out = np.zeros((T, B, H), np.float32)
    for core, rmap in enumerate(res.results):
        bh, j = core // N_CHUNKS, core % N_CHUNKS
        bsl = slice(bh * BL, (bh + 1) * BL)
        o = np.asarray(rmap["out"])  # [S, 128, 4*BL] bf16
        # o[i, p, c*BL+b] = h3[b, c*128+p]
        o = np.asarray(o, dtype=np.float32).reshape(S, 128, 4, BL)
        o = o.transpose(0, 3, 2, 1).reshape(S, BL, H)
        if j == 0:
            out[0:S, bsl] = o
        else:
            lo = chunk_starts[j] + WARM
            out[lo:lo + (S - WARM), bsl] = o[WARM:]
    return out


# revision 20
# speedup vs baseline: 1.0743x; 1.0743x over previous
"""Trainium2 Bass kernel: Conv2d(1->64,3x3) + 3-layer GRU over T=256.

Weight-stationary formulation (v2):
  - Conv folded into layer-0 input weights host-side (gi0[t] = W_eff @ x3[t]).
  - 8 cores = 2 batch halves (32 each) x 4 time chunks with WARM-step warmup
    (GRU state contracts ~0.77/step, so warm chunks converge).
  - All GEMMs keep the WEIGHTS as the stationary operand and h as the moving
    operand, producing gates in [gate, batch] layout: each matmul streams only
    BL=32 columns, every elementwise op runs 128 partitions wide, and h never
    needs transposing (h' is produced directly in the layout the next step's
    matmuls consume).
  - Per (layer, step): one PSUM bank [128, 512] holds 16 blocks of 32 cols:
    blocks 0..7 = r|z pre-acts (gi+gh+bias), 8..11 = gi_n, 12..15 = gh_n.
    Biases enter via a single K=16 selector matmul that writes the whole bank
    (start=True), then gi/gh chunk matmuls accumulate block-wise.
  - Eltwise: sigmoid (ACT, psum->sbuf bf16), n = tanh(gi_n + r*gh_n) (DVE mul/
    add + ACT tanh), h' = n + z*(h-n) (DVE bf16). h state lives in bf16.
  - Wavefront: span s runs (l=0,t=s), (l=1,t=s-1), (l=2,t=s-2); all cross-
    group deps come from the previous span, so groups in a span are
    independent and the eltwise chain of one group hides under the PE stream
    of the others.
  - Single-sync-wait discipline (walrus limit): cheap absorber instructions
    (PE ldweights / 1-elem DVE copies / Pool memsets) carry all but one of
    each real instruction's cross-engine waits; Tile's wait assignment then
    elides the dominated ones.
  - Weights/bias/x3 stream in via SP-issued (HWDGE) DMAs; per-step output
    DMAs via gpsimd (SWDGE) read the layer-2 h tile directly.
"""

import sys

for _p in ("/opt/trn_rl_repo",):
    if _p not in sys.path:
        sys.path.insert(0, _p)

import numpy as np
import ml_dtypes

import concourse.bass as bass
import concourse.mybir as mybir
import concourse.tile as tile
from concourse.bass import _add_dep_helper
from concourse.bass_utils import run_bass_kernel_spmd

BF16 = mybir.dt.bfloat16
F32 = mybir.dt.float32
AF = mybir.ActivationFunctionType

B, NB, T, F, H = 64, 64, 256, 64, 512
G3 = 3 * H  # 1536
KX = 3 * NB  # 192 folded-conv contraction
BATCH_WAYS = 2
N_CHUNKS = 4
WARM = 24
S = (T + (N_CHUNKS - 1) * WARM) // N_CHUNKS  # 88 steps per core
BL = B // BATCH_WAYS  # 32 batch rows per core
NT = 12  # gate tiles of 128 (1536/128)

_NC_CACHE: dict = {}
_DBG_LABELS: dict = {}  # inst name -> semantic label (debug aid)


def _lbl(h, label):
    _DBG_LABELS[h.ins.name] = label
    return h


def _build_nc(s_steps: int = S, bl: int = BL):
    nc = bass.Bass()

    wg0_ext = nc.declare_dram_parameter("wg0", [128, 2 * G3], BF16, isOutput=False)
    wg12_ext = nc.declare_dram_parameter("wg12", [2, 128, 4 * G3], BF16,
                                         isOutput=False)
    wh_ext = nc.declare_dram_parameter("wh", [3, 128, 4 * G3], BF16, isOutput=False)
    bmat_ext = nc.declare_dram_parameter("bmat", [16, 3 * 128], BF16, isOutput=False)
    sel_ext = nc.declare_dram_parameter("sel", [16, 512], BF16, isOutput=False)
    x3t_ext = nc.declare_dram_parameter("x3t", [s_steps, 128, 64], BF16, isOutput=False)
    h0t_ext = nc.declare_dram_parameter("h0t", [3, 128, 4 * bl], BF16, isOutput=False)
    out_ext = nc.declare_dram_parameter("out", [s_steps, 128, 4 * bl], BF16,
                                        isOutput=True)

    from contextlib import ExitStack

    with tile.TileContext(nc) as tc, ExitStack() as ctx:
        wpool = ctx.enter_context(tc.tile_pool(name="static", bufs=1))
        hT_pools = [
            ctx.enter_context(tc.tile_pool(name=f"hT{l}", bufs=(3, 6, 8)[l]))
            for l in range(3)
        ]
        rz_pools = [ctx.enter_context(tc.tile_pool(name=f"rz{l}", bufs=3))
                    for l in range(3)]
        nmul_pools = [ctx.enter_context(tc.tile_pool(name=f"nmul{l}", bufs=3))
                      for l in range(3)]
        npre_pools = [ctx.enter_context(tc.tile_pool(name=f"npre{l}", bufs=3))
                      for l in range(3)]
        nt_pools = [ctx.enter_context(tc.tile_pool(name=f"nt{l}", bufs=3))
                    for l in range(3)]
        d_pools = [ctx.enter_context(tc.tile_pool(name=f"d{l}", bufs=3))
                   for l in range(3)]
        zd_pools = [ctx.enter_context(tc.tile_pool(name=f"zd{l}", bufs=3))
                    for l in range(3)]
        ps_pools = [
            ctx.enter_context(tc.tile_pool(name=f"ps{l}", bufs=2, space="PSUM"))
            for l in range(3)
        ]

        # ---- static SBUF slabs ------------------------------------------
        wg0_sb = wpool.tile([128, 2 * G3], BF16, tag="wg0")
        wg12_sb = wpool.tile([128, 8 * G3], BF16, tag="wg12")
        wh_sb = wpool.tile([128, 12 * G3], BF16, tag="wh")
        bmat_sb = wpool.tile([128, 3 * 128], BF16, tag="bmat")
        sel_sb = wpool.tile([128, 512], BF16, tag="sel")
        x3_sb = wpool.tile([128, s_steps * 64], BF16, tag="x3")
        h0_sb = wpool.tile([128, 3 * 4 * bl], BF16, tag="h0")
        dummy = wpool.tile([1, 2048], F32, tag="dummy")
        act_dummy = wpool.tile([1, 2048], F32, tag="actdummy")
        dummy_ctr = [0]
        act_ctr = [0]

        # ---- preamble DMAs (SP / HWDGE) ---------------------------------
        sp_dmas = []

        def _sdma(dst, src):
            d = nc.sync.dma_start(dst, src)
            sp_dmas.append(d)
            return d

        wg_last = [None, None, None]  # last DMA per layer's gi slab
        wh_last = [None, None, None]
        x3_dma = [None] * s_steps

        def _x3range(lo, hi):
            for i in range(lo, min(hi, s_steps)):
                x3_dma[i] = _sdma(x3_sb[:, i * 64:(i + 1) * 64], x3t_ext[i])

        bmat_dma = _sdma(bmat_sb[0:16, :], bmat_ext[:, :])
        sel_dma = _sdma(sel_sb[0:16, :], sel_ext[:, :])
        _x3range(0, 2)
        wg_last[0] = _sdma(wg0_sb[:, :], wg0_ext[:, :])
        wh_last[0] = _sdma(wh_sb[:, 0:4 * G3], wh_ext[0])
        h0_dma = [None] * 3
        for l in range(3):
            h0_dma[l] = _sdma(h0_sb[:, l * 4 * bl:(l + 1) * 4 * bl], h0t_ext[l])
        _x3range(2, 6)
        wg_last[1] = _sdma(wg12_sb[:, 0:4 * G3], wg12_ext[0])
        wh_last[1] = _sdma(wh_sb[:, 4 * G3:8 * G3], wh_ext[1])
        _x3range(6, 10)
        wg_last[2] = _sdma(wg12_sb[:, 4 * G3:8 * G3], wg12_ext[1])
        wh_last[2] = _sdma(wh_sb[:, 8 * G3:12 * G3], wh_ext[2])
        _x3range(10, s_steps)

        # PE absorbers so the first bias matmul never carries DMA waits
        for dep in (bmat_dma, sel_dma):
            a = nc.tensor.ldweights(bmat_sb[0:1, 0:1])
            _add_dep_helper(a.ins, dep.ins, sync=True, reason="preamble prime")

        # ---- bookkeeping -------------------------------------------------
        hT = [dict() for _ in range(3)]     # (l, i) -> h tile [128, 4*bl]
        hprod = [dict() for _ in range(3)]  # (l, i) -> producing instruction
        sig_hist = [dict() for _ in range(3)]
        tanh_hist = [dict() for _ in range(3)]
        add1_hist = [dict() for _ in range(3)]
        out_dmas = []
        last_eng = {}

        for l in range(3):
            hT[l][-1] = h0_sb[:, l * 4 * bl:(l + 1) * 4 * bl]

        def ldw_abs(dep, reason):
            a = nc.tensor.ldweights(bmat_sb[0:1, 0:1])
            _add_dep_helper(a.ins, dep.ins, sync=True, reason=reason)
            return a

        def dve_abs_read(src_ap):
            c = dummy_ctr[0] % 2048
            dummy_ctr[0] += 1
            return nc.vector.tensor_copy(dummy[0:1, c:c + 1], src_ap)

        act_zero = nc.const_aps.scalar_like(0.0, act_dummy[0:1, 0:1])

        def act_abs(dep, reason):
            c = act_ctr[0] % 2048
            act_ctr[0] += 1
            a = nc.scalar.activation(act_dummy[0:1, c:c + 1], act_zero, AF.Copy)
            _add_dep_helper(a.ins, dep.ins, sync=True, reason=reason)
            return a

        def emit_group(l, i):
            # --- wait absorbers (keep every real instruction at <=1 wait) --
            grp_abs = []
            if i == 0:
                grp_abs.append(ldw_abs(wg_last[l], f"wg{l} slab ready"))
                grp_abs.append(ldw_abs(wh_last[l], f"wh{l} slab ready"))
                grp_abs.append(ldw_abs(h0_dma[l], "h0 slab ready"))
            else:
                grp_abs.append(
                    ldw_abs(hprod[l][i - 1], "h[l][i-1] ready (covers h[l-1][i])"))
                if l >= 1:
                    grp_abs.append(
                        ldw_abs(hprod[l - 1][i], "h[l-1][i] ready"))
            if i >= 2:
                grp_abs.append(ldw_abs(sig_hist[l][i - 2], "psum WAR vs old sig"))
                grp_abs.append(ldw_abs(add1_hist[l][i - 2], "psum WAR vs old add1"))

            ps = ps_pools[l].tile([128, 512], F32, tag=f"ps{l}")

            # --- matmuls --------------------------------------------------
            # bias: psum[p, c] = bmat[c//32, p] for the whole bank
            mm_bias = _lbl(nc.tensor.matmul(
                ps[:, :], bmat_sb[0:16, l * 128:(l + 1) * 128],
                sel_sb[0:16, :], start=True, stop=False,
                skip_group_check=True), f"mm_bias l{l} i{i}")
            for a in grp_abs:
                _add_dep_helper(mm_bias.ins, a.ins, sync=False,
                                reason="group absorbers precede first matmul")

            if l == 0:
                gi_src = [(x3_sb[0:128, i * 64:i * 64 + 32], 0),
                          (x3_sb[0:64, i * 64 + 32:i * 64 + 64], 1)]

                def gi_w(c, j):
                    return wg0_sb[0:(128 if c == 0 else 64),
                                  c * G3 + j * 128:c * G3 + (j + 1) * 128]
            else:
                hsrc = hT[l - 1][i]
                gi_src = [(hsrc[:, c * bl:(c + 1) * bl], c) for c in range(4)]

                def gi_w(c, j):
                    base = ((l - 1) * 4 + c) * G3
                    return wg12_sb[:, base + j * 128:base + (j + 1) * 128]

            ghsrc = hT[l][i - 1]
            gh_src = [(ghsrc[:, c * bl:(c + 1) * bl], c) for c in range(4)]

            def gh_w(c, j):
                base = (l * 4 + c) * G3
                return wh_sb[:, base + j * 128:base + (j + 1) * 128]

            mms = []
            # r|z blocks first (sigmoid can start mid-stream), then gi_n,
            # then gh_n last (mul1's single PE wait lands on the final MM)
            for j in range(0, 8):
                for rhs, c in gi_src:
                    mms.append((gi_w(c, j), rhs, j))
                for rhs, c in gh_src:
                    mms.append((gh_w(c, j), rhs, j))
            for j in range(8, 12):
                for rhs, c in gi_src:
                    mms.append((gi_w(c, j), rhs, j))
            for j in range(8, 12):
                for rhs, c in gh_src:
                    mms.append((gh_w(c, j), rhs, j + 4))
            n = len(mms)
            prev_mm = mm_bias
            for idx, (w, rhs, blk) in enumerate(mms):
                kk = w.shape[0]
                prev_mm = _lbl(nc.tensor.matmul(
                    ps[:, blk * bl:(blk + 1) * bl], w, rhs[0:kk, :],
                    start=False, stop=(idx == n - 1), skip_group_check=True),
                    f"mm l{l} i{i} #{idx} blk{blk}")
            last_eng['PE'] = prev_mm

            # --- eltwise --------------------------------------------------
            # r|z sigmoid -> sbuf bf16
            rz = rz_pools[l].tile([128, 2 * 4 * bl], BF16, tag=f"rz{l}")
            if i >= 3:
                sa = act_abs(sig_hist[l][i - 3], "rz WAW completion")
            sig = _lbl(nc.scalar.activation(rz[:, :], ps[:, 0:8 * bl], AF.Sigmoid),
                       f"sig l{l} i{i}")
            if i >= 3:
                _add_dep_helper(sig.ins, sa.ins, sync=False,
                                reason="WAW absorber precedes sigmoid")
            sig_hist[l][i] = sig

            # absorb the sigmoid tick into the DVE clock
            dabs = _lbl(dve_abs_read(rz[0:1, 0:1]), f"dve_abs_sig l{l} i{i}")
            pre_mul = [dabs]
            if i == 0:
                pre_mul.append(_lbl(
                    dve_abs_read(h0_sb[0:1, l * 4 * bl:l * 4 * bl + 1]),
                    f"dve_abs_h0 l{l}"))
            # n = tanh(gi_n + r * gh_n)
            nm = nmul_pools[l].tile([128, 4 * bl], F32, tag=f"nmul{l}")
            mul1 = _lbl(nc.vector.tensor_mul(nm[:, :], rz[:, 0:4 * bl],
                                             ps[:, 12 * bl:16 * bl]),
                        f"mul1 l{l} i{i}")
            for a in pre_mul:
                _add_dep_helper(mul1.ins, a.ins, sync=False,
                                reason="absorbers precede n-path mult")
            npre = npre_pools[l].tile([128, 4 * bl], F32, tag=f"npre{l}")
            add1 = _lbl(nc.vector.tensor_add(npre[:, :], nm[:, :],
                                             ps[:, 8 * bl:12 * bl]),
                        f"add1 l{l} i{i}")
            add1_hist[l][i] = add1
            ntl = nt_pools[l].tile([128, 4 * bl], BF16, tag=f"nt{l}")
            if i >= 3:
                ta = act_abs(tanh_hist[l][i - 3], "nt WAW completion")
            tanh = _lbl(nc.scalar.activation(ntl[:, :], npre[:, :], AF.Tanh),
                        f"tanh l{l} i{i}")
            if i >= 3:
                _add_dep_helper(tanh.ins, ta.ins, sync=False,
                                reason="WAW absorber precedes tanh")
            tanh_hist[l][i] = tanh
            last_eng['ACT'] = tanh

            # h' = n + z*(h - n)   (all bf16, SBUF)
            dt_ = d_pools[l].tile([128, 4 * bl], BF16, tag=f"d{l}")
            sub = _lbl(nc.vector.tensor_sub(dt_[:, :], ghsrc[:, :], ntl[:, :]),
                       f"sub l{l} i{i}")
            zd = zd_pools[l].tile([128, 4 * bl], BF16, tag=f"zd{l}")
            zdm = _lbl(nc.vector.tensor_mul(zd[:, :], rz[:, 4 * bl:8 * bl],
                                            dt_[:, :]), f"zd l{l} i{i}")
            pre_hp = []
            if l == 2 and i >= 8:
                # hT2 slot recycle: absorb the old out-DMA tick into DVE
                a = _lbl(dve_abs_read(nc.const_aps.scalar_like(0.0, dummy[0:1, 0:1])), f"dve_abs_odma i{i}")
                _add_dep_helper(a.ins, out_dmas[i - 8].ins, sync=True,
                                reason="hT2 WAR vs old out DMA")
                pre_hp.append(a)
            hnew = hT_pools[l].tile([128, 4 * bl], BF16, tag=f"hT{l}")
            hp = _lbl(nc.vector.tensor_add(hnew[:, :], zd[:, :], ntl[:, :]),
                      f"hp l{l} i{i}")
            for a in pre_hp:
                _add_dep_helper(hp.ins, a.ins, sync=False,
                                reason="out-DMA absorber precedes h'")
            last_eng['DVE'] = hp
            hT[l][i] = hnew
            hprod[l][i] = hp
            if i - 5 in hT[l] and l != 2:
                del hT[l][i - 5]

            if l == 2:
                prev_dma = None
                if len(out_dmas) >= 8:
                    c = dummy_ctr[0] % 2048
                    dummy_ctr[0] += 1
                    m = _lbl(nc.gpsimd.memset(dummy[0:1, c:c + 1], 0.0),
                             f"pool_abs i{i}")
                    _add_dep_helper(m.ins, out_dmas[-8].ins, sync=True,
                                    reason="SWDGE ring slot")
                    last_eng['POOL'] = m
                    prev_dma = m
                dma = _lbl(nc.gpsimd.dma_start(out_ext[i], hnew[:, :]),
                           f"out_dma i{i}")
                if prev_dma is not None:
                    _add_dep_helper(dma.ins, prev_dma.ins, sync=False,
                                    reason="ring absorber precedes out DMA")
                out_dmas.append(dma)
                if i - 9 in hT[2]:
                    del hT[2][i - 9]

        LAG = (0, 1, 2)
        for s in range(s_steps + LAG[2]):
            for l in range(3):
                i = s - LAG[l]
                if 0 <= i < s_steps:
                    emit_group(l, i)

        # ---- tail pre-drains (final Drain must see <=1 new wait) ---------
        tail = list(last_eng.values()) + out_dmas[-8:] + sp_dmas[-8:]
        for dep in tail:
            dr = nc.sync.drain(fusable=False)
            _add_dep_helper(dr.ins, dep.ins, sync=True, reason="tail pre-drain")

    return nc


# ---------------------------------------------------------------------------
# Host-side input preparation


def _fold_conv(conv_w, conv_b, w_ih0, b_ih0):
    """Fold conv into layer0 input weights: gi0[t] = W_eff @ x3[t] + b_eff."""
    RNN_IN = F * (NB - 2)
    C = np.zeros((RNN_IN, KX), np.float64)
    for f in range(F):
        for di in range(3):
            for dt in range(3):
                w = float(conv_w[f, 0, di, dt])
                for i in range(NB - 2):
                    C[f * (NB - 2) + i, dt * NB + (i + di)] += w
    W_eff = w_ih0.astype(np.float64) @ C  # [1536, 192]
    bc = np.repeat(conv_b.astype(np.float64), NB - 2)
    b_eff = b_ih0.astype(np.float64) + w_ih0.astype(np.float64) @ bc
    return W_eff.astype(np.float32), b_eff.astype(np.float32)


def _bf16(a):
    return np.ascontiguousarray(a.astype(ml_dtypes.bfloat16))


def _prep_core_inputs(inputs, s_steps=S, warm=WARM):
    x = np.asarray(inputs["x"], np.float32)
    W_eff, b_eff = _fold_conv(np.asarray(inputs["conv_w"], np.float32),
                              np.asarray(inputs["conv_b"], np.float32),
                              np.asarray(inputs["w_ih0"], np.float32),
                              np.asarray(inputs["b_ih0"], np.float32))

    def slab4(wT):
        # [512, 1536] -> [128, 4*1536] slab image (chunk-major cols)
        return wT.reshape(4, 128, G3).transpose(1, 0, 2).reshape(128, 4 * G3)

    wg0 = np.zeros((128, 2 * G3), np.float32)
    WeT = W_eff.T  # [192, 1536]
    wg0[:, 0:G3] = WeT[0:128]
    wg0[0:64, G3:2 * G3] = WeT[128:192]
    wg12 = np.zeros((2, 128, 4 * G3), np.float32)
    for l in (1, 2):
        wg12[l - 1] = slab4(np.asarray(inputs[f"w_ih{l}"], np.float32).T)
    wh = np.zeros((3, 128, 4 * G3), np.float32)
    for l in range(3):
        wh[l] = slab4(np.asarray(inputs[f"w_hh{l}"], np.float32).T)

    bmat = np.zeros((16, 3 * 128), np.float32)
    for l in range(3):
        b_h = np.asarray(inputs[f"b_hh{l}"], np.float32)
        b_i = b_eff if l == 0 else np.asarray(inputs[f"b_ih{l}"], np.float32)
        col = slice(l * 128, (l + 1) * 128)
        both = b_i + b_h
        for k in range(8):
            bmat[k, col] = both[k * 128:(k + 1) * 128]
        for k in range(4):
            bmat[8 + k, col] = b_i[1024 + k * 128:1024 + (k + 1) * 128]
            bmat[12 + k, col] = b_h[1024 + k * 128:1024 + (k + 1) * 128]
    sel = np.zeros((16, 512), np.float32)
    for k in range(16):
        sel[k, k * 32:(k + 1) * 32] = 1.0

    wg0_b, wg12_b, wh_b = _bf16(wg0), _bf16(wg12), _bf16(wh)
    bmat_b, sel_b = _bf16(bmat), _bf16(sel)

    x2p = np.pad(x[:, 0], ((0, 0), (0, 0), (1, 1)))  # [B, NB, T+2]
    hs = [np.asarray(inputs[f"h{l + 1}"], np.float32) for l in range(3)]

    in_maps = []
    chunk_starts = [0] + [s_steps + (j - 1) * (s_steps - warm) - warm
                          for j in range(1, N_CHUNKS)]
    for bh in range(BATCH_WAYS):
        bsl = slice(bh * BL, (bh + 1) * BL)
        for j in range(N_CHUNKS):
            t0 = chunk_starts[j]
            seg = x2p[bsl, :, t0:t0 + s_steps + 2]  # [BL, 64, S+2]
            A = np.stack([seg[:, :, dt:dt + s_steps] for dt in range(3)], axis=0)
            w3T = A.transpose(0, 2, 1, 3).reshape(KX, BL, s_steps)  # [k, b, i]
            x3t = np.zeros((s_steps, 128, 64), np.float32)
            x3t[:, :, 0:32] = w3T[0:128].transpose(2, 0, 1)
            x3t[:, 0:64, 32:64] = w3T[128:192].transpose(2, 0, 1)
            h0t = np.zeros((3, 128, 4 * BL), np.float32)
            if j == 0:
                for l in range(3):
                    hT0 = hs[l][bsl].T  # [H, BL]
                    for c in range(4):
                        h0t[l, :, c * BL:(c + 1) * BL] = hT0[c * 128:(c + 1) * 128]
            in_maps.append({
                "wg0": wg0_b, "wg12": wg12_b, "wh": wh_b,
                "bmat": bmat_b, "sel": sel_b,
                "x3t": _bf16(x3t), "h0t": _bf16(h0t),
            })
    return in_maps, chunk_starts


def kernel(**inputs) -> np.ndarray:
    if "nc" not in _NC_CACHE:
        _NC_CACHE["nc"] = _build_nc()
    nc = _NC_CACHE["nc"]
    in_maps, chunk_starts = _prep_core_inputs(inputs)
    res = run_bass_kernel_spmd(nc, in_maps, list(range(8)))
    _NC_CACHE["last_result"] = res
    out = np.zeros((T, B, H), np.float32)
    for core, rmap in enumerate(res.results):
        bh, j = core // N_CHUNKS, core % N_CHUNKS
        bsl = slice(bh * BL, (bh + 1) * BL)
        o = np.asarray(rmap["out"])  # [S, 128, 4*BL] bf16
        # o[i, p, c*BL+b] = h3[b, c*128+p]
        o = np.asarray(o, dtype=np.float32).reshape(S, 128, 4, BL)
        o = o.transpose(0, 3, 2, 1).reshape(S, BL, H)
        if j == 0:
            out[0:S, bsl] = o
        else:
            lo = chunk_starts[j] + WARM
            out[lo:lo + (S - WARM), bsl] = o[WARM:]
    return out


# revision 26
# speedup vs baseline: 1.1220x; 1.0444x over previous
"""Trainium2 Bass kernel: Conv2d(1->64,3x3) + 3-layer GRU over T=256.

Weight-stationary formulation (v2):
  - Conv folded into layer-0 input weights host-side (gi0[t] = W_eff @ x3[t]).
  - 8 cores = 2 batch halves (32 each) x 4 time chunks with WARM-step warmup
    (GRU state contracts ~0.77/step, so warm chunks converge).
  - All GEMMs keep the WEIGHTS as the stationary operand and h as the moving
    operand, producing gates in [gate, batch] layout: each matmul streams only
    BL=32 columns, every elementwise op runs 128 partitions wide, and h never
    needs transposing (h' is produced directly in the layout the next step's
    matmuls consume).
  - Per (layer, step): one PSUM bank [128, 512] holds 16 blocks of 32 cols:
    blocks 0..7 = r|z pre-acts (gi+gh+bias), 8..11 = gi_n, 12..15 = gh_n.
    Biases enter via a single K=16 selector matmul that writes the whole bank
    (start=True), then gi/gh chunk matmuls accumulate block-wise.
  - Eltwise: sigmoid (ACT, psum->sbuf bf16), n = tanh(gi_n + r*gh_n) (DVE mul/
    add + ACT tanh), h' = n + z*(h-n) (DVE bf16). h state lives in bf16.
  - Wavefront: span s runs (l=0,t=s), (l=1,t=s-1), (l=2,t=s-2); all cross-
    group deps come from the previous span, so groups in a span are
    independent and the eltwise chain of one group hides under the PE stream
    of the others.
  - Single-sync-wait discipline (walrus limit): cheap absorber instructions
    (PE ldweights / 1-elem DVE copies / Pool memsets) carry all but one of
    each real instruction's cross-engine waits; Tile's wait assignment then
    elides the dominated ones.
  - Weights/bias/x3 stream in via SP-issued (HWDGE) DMAs; per-step output
    DMAs via gpsimd (SWDGE) read the layer-2 h tile directly.
"""

import sys

for _p in ("/opt/trn_rl_repo",):
    if _p not in sys.path:
        sys.path.insert(0, _p)

import numpy as np
import ml_dtypes

import concourse.bass as bass
import concourse.mybir as mybir
import concourse.tile as tile
from concourse.bass import _add_dep_helper
from concourse.bass_utils import run_bass_kernel_spmd

BF16 = mybir.dt.bfloat16
F32 = mybir.dt.float32
AF = mybir.ActivationFunctionType

B, NB, T, F, H = 64, 64, 256, 64, 512
G3 = 3 * H  # 1536
KX = 3 * NB  # 192 folded-conv contraction
BATCH_WAYS = 2
N_CHUNKS = 4
WARM = 24
S = (T + (N_CHUNKS - 1) * WARM) // N_CHUNKS  # 88 steps per core
BL = B // BATCH_WAYS  # 32 batch rows per core
NT = 12  # gate tiles of 128 (1536/128)

_NC_CACHE: dict = {}
_DBG_LABELS: dict = {}  # inst name -> semantic label (debug aid)


def _lbl(h, label):
    _DBG_LABELS[h.ins.name] = label
    return h


def _build_nc(s_steps: int = S, bl: int = BL):
    nc = bass.Bass()

    wg0_ext = nc.declare_dram_parameter("wg0", [128, 2 * G3], BF16, isOutput=False)
    wg12_ext = nc.declare_dram_parameter("wg12", [2, 128, 4 * G3], BF16,
                                         isOutput=False)
    wh_ext = nc.declare_dram_parameter("wh", [3, 128, 4 * G3], BF16, isOutput=False)
    bmat_ext = nc.declare_dram_parameter("bmat", [16, 512], BF16, isOutput=False)
    sel_ext = nc.declare_dram_parameter("sel", [16, 640], BF16, isOutput=False)
    x3t_ext = nc.declare_dram_parameter("x3t", [128, s_steps * 64], BF16,
                                       isOutput=False)
    h0t_ext = nc.declare_dram_parameter("h0t", [3, 128, 4 * bl], BF16, isOutput=False)
    out_ext = nc.declare_dram_parameter("out", [s_steps, 128, 4 * bl], BF16,
                                        isOutput=True)

    from contextlib import ExitStack

    with tile.TileContext(nc) as tc, ExitStack() as ctx:
        wpool = ctx.enter_context(tc.tile_pool(name="static", bufs=1))
        hT_pools = [
            ctx.enter_context(tc.tile_pool(name=f"hT{l}", bufs=(3, 6, 8)[l]))
            for l in range(3)
        ]
        rz_pools = [ctx.enter_context(tc.tile_pool(name=f"rz{l}", bufs=3))
                    for l in range(3)]
        nmul_pools = [ctx.enter_context(tc.tile_pool(name=f"nmul{l}", bufs=3))
                      for l in range(3)]
        npre_pools = [ctx.enter_context(tc.tile_pool(name=f"npre{l}", bufs=3))
                      for l in range(3)]
        nt_pools = [ctx.enter_context(tc.tile_pool(name=f"nt{l}", bufs=3))
                    for l in range(3)]
        d_pools = [ctx.enter_context(tc.tile_pool(name=f"d{l}", bufs=3))
                   for l in range(3)]
        zd_pools = [ctx.enter_context(tc.tile_pool(name=f"zd{l}", bufs=3))
                    for l in range(3)]
        ps_pools = [
            ctx.enter_context(tc.tile_pool(name=f"ps{l}", bufs=2, space="PSUM"))
            for l in range(3)
        ]

        # ---- static SBUF slabs ------------------------------------------
        wg0_sb = wpool.tile([128, 2 * G3], BF16, tag="wg0")
        wg12_sb = wpool.tile([128, 8 * G3], BF16, tag="wg12")
        wh_sb = wpool.tile([128, 12 * G3], BF16, tag="wh")
        bmat_sb = wpool.tile([128, 512], BF16, tag="bmat")
        sel_sb = wpool.tile([128, 640], BF16, tag="sel")
        x3_sb = wpool.tile([128, s_steps * 64], BF16, tag="x3")
        h0_sb = wpool.tile([128, 3 * 4 * bl], BF16, tag="h0")
        dummy = wpool.tile([1, 2048], F32, tag="dummy")
        act_dummy = wpool.tile([1, 2048], F32, tag="actdummy")
        dummy_ctr = [0]
        act_ctr = [0]

        # ---- preamble DMAs (SP / HWDGE) ---------------------------------
        sp_dmas = []

        def _sdma(dst, src):
            d = nc.sync.dma_start(dst, src)
            sp_dmas.append(d)
            return d

        wg_last = [None, None, None]  # last DMA per layer's gi slab
        wh_last = [None, None, None]
        x3_dma = [None] * s_steps

        bmat_dma = _sdma(bmat_sb[0:16, :], bmat_ext[:, :])
        sel_dma = _sdma(sel_sb[0:16, :], sel_ext[:, :])
        X3A = 8  # steps in the first (startup-critical) x3 image
        d = _sdma(x3_sb[:, 0:X3A * 64], x3t_ext[:, 0:X3A * 64])
        for i in range(X3A):
            x3_dma[i] = d
        h0_dma = [None] * 3
        for l in range(3):
            h0_dma[l] = _sdma(h0_sb[:, l * 4 * bl:(l + 1) * 4 * bl], h0t_ext[l])
        # chunk-granular first-layer weight DMAs so span 0's matmuls start
        # as each chunk lands
        _sdma(wg0_sb[:, 0:G3], wg0_ext[:, 0:G3])
        wg_last[0] = _sdma(wg0_sb[:, G3:2 * G3], wg0_ext[:, G3:2 * G3])
        for c in range(4):
            wh_last[0] = _sdma(wh_sb[:, c * G3:(c + 1) * G3],
                               wh_ext[0, :, c * G3:(c + 1) * G3])
        wg_last[1] = _sdma(wg12_sb[:, 0:4 * G3], wg12_ext[0])
        wh_last[1] = _sdma(wh_sb[:, 4 * G3:8 * G3], wh_ext[1])
        wg_last[2] = _sdma(wg12_sb[:, 4 * G3:8 * G3], wg12_ext[1])
        wh_last[2] = _sdma(wh_sb[:, 8 * G3:12 * G3], wh_ext[2])
        d = _sdma(x3_sb[:, X3A * 64:], x3t_ext[:, X3A * 64:])
        for i in range(X3A, s_steps):
            x3_dma[i] = d

        # PE absorbers so the first bias matmul never carries DMA waits
        for dep in (bmat_dma, sel_dma):
            a = nc.tensor.ldweights(bmat_sb[0:1, 0:1])
            _add_dep_helper(a.ins, dep.ins, sync=True, reason="preamble prime")

        # ---- bookkeeping -------------------------------------------------
        hT = [dict() for _ in range(3)]     # (l, i) -> h tile [128, 4*bl]
        hprod = [dict() for _ in range(3)]  # (l, i) -> producing instruction
        sig_hist = [dict() for _ in range(3)]
        tanh_hist = [dict() for _ in range(3)]
        add1_hist = [dict() for _ in range(3)]
        out_dmas = []
        last_eng = {}

        for l in range(3):
            hT[l][-1] = h0_sb[:, l * 4 * bl:(l + 1) * 4 * bl]

        def ldw_abs(dep, reason):
            a = nc.tensor.ldweights(bmat_sb[0:1, 0:1])
            _add_dep_helper(a.ins, dep.ins, sync=True, reason=reason)
            return a

        def dve_abs_read(src_ap):
            c = dummy_ctr[0] % 2048
            dummy_ctr[0] += 1
            return nc.vector.tensor_copy(dummy[0:1, c:c + 1], src_ap)

        act_zero = nc.const_aps.scalar_like(0.0, act_dummy[0:1, 0:1])

        def act_abs(dep, reason):
            c = act_ctr[0] % 2048
            act_ctr[0] += 1
            a = nc.scalar.activation(act_dummy[0:1, c:c + 1], act_zero, AF.Copy)
            _add_dep_helper(a.ins, dep.ins, sync=True, reason=reason)
            return a

        def emit_group(l, i):
            # --- wait absorbers (keep every real instruction at <=1 wait) --
            grp_abs = []
            if i == 0:
                grp_abs.append(ldw_abs(wg_last[l], f"wg{l} slab ready"))
                grp_abs.append(ldw_abs(wh_last[l], f"wh{l} slab ready"))
                grp_abs.append(ldw_abs(h0_dma[l], "h0 slab ready"))
            else:
                grp_abs.append(
                    ldw_abs(hprod[l][i - 1], "h[l][i-1] ready (covers h[l-1][i])"))
                if l >= 1:
                    grp_abs.append(
                        ldw_abs(hprod[l - 1][i], "h[l-1][i] ready"))
            if i >= 2:
                grp_abs.append(ldw_abs(sig_hist[l][i - 2], "psum WAR vs old sig"))
                grp_abs.append(ldw_abs(add1_hist[l][i - 2], "psum WAR vs old add1"))

            ps = ps_pools[l].tile([128, 512], F32, tag=f"ps{l}")

            # --- matmuls --------------------------------------------------
            # bias: psum[p, c] = bmat[c//32, p]. start=True marks the whole
            # bank pending-zero, so later start=False matmuls overwrite-on-
            # first-touch. l0 r|z|gin biases ride x3's ones row instead, so
            # its bias matmul only covers the gh_n blocks.
            if l == 0:
                mm_bias = _lbl(nc.tensor.matmul(
                    ps[:, 12 * bl:16 * bl], bmat_sb[0:4, 384:512],
                    sel_sb[0:4, 512:640], start=True, stop=False,
                    skip_group_check=True), f"mm_bias l{l} i{i}")
            else:
                mm_bias = _lbl(nc.tensor.matmul(
                    ps[:, :], bmat_sb[0:16, l * 128:(l + 1) * 128],
                    sel_sb[0:16, 0:512], start=True, stop=False,
                    skip_group_check=True), f"mm_bias l{l} i{i}")
            for a in grp_abs:
                _add_dep_helper(mm_bias.ins, a.ins, sync=False,
                                reason="group absorbers precede first matmul")

            if l == 0:
                gi_src = [(x3_sb[0:128, i * 64:i * 64 + 32], 0),
                          (x3_sb[0:65, i * 64 + 32:i * 64 + 64], 1)]

                def gi_w(c, j):
                    return wg0_sb[0:(128 if c == 0 else 65),
                                  c * G3 + j * 128:c * G3 + (j + 1) * 128]
            else:
                hsrc = hT[l - 1][i]
                gi_src = [(hsrc[:, c * bl:(c + 1) * bl], c) for c in range(4)]

                def gi_w(c, j):
                    base = ((l - 1) * 4 + c) * G3
                    return wg12_sb[:, base + j * 128:base + (j + 1) * 128]

            ghsrc = hT[l][i - 1]
            gh_src = [(ghsrc[:, c * bl:(c + 1) * bl], c) for c in range(4)]

            def gh_w(c, j):
                base = (l * 4 + c) * G3
                return wh_sb[:, base + j * 128:base + (j + 1) * 128]

            mms = []
            # r|z blocks first (sigmoid can start mid-stream), then gi_n,
            # then gh_n last (mul1's single PE wait lands on the final MM)
            for j in range(0, 8):
                for rhs, c in gi_src:
                    mms.append((gi_w(c, j), rhs, j))
                for rhs, c in gh_src:
                    mms.append((gh_w(c, j), rhs, j))
            for j in range(8, 12):
                for rhs, c in gi_src:
                    mms.append((gi_w(c, j), rhs, j))
            for j in range(8, 12):
                for rhs, c in gh_src:
                    mms.append((gh_w(c, j), rhs, j + 4))
            n = len(mms)
            prev_mm = mm_bias
            for idx, (w, rhs, blk) in enumerate(mms):
                kk = w.shape[0]
                prev_mm = _lbl(nc.tensor.matmul(
                    ps[:, blk * bl:(blk + 1) * bl], w, rhs[0:kk, :],
                    start=False, stop=(idx == n - 1), skip_group_check=True),
                    f"mm l{l} i{i} #{idx} blk{blk}")
            last_eng['PE'] = prev_mm

            # --- eltwise --------------------------------------------------
            # r|z sigmoid -> sbuf bf16
            rz = rz_pools[l].tile([128, 2 * 4 * bl], BF16, tag=f"rz{l}")
            if i >= 3:
                sa = act_abs(sig_hist[l][i - 3], "rz WAW completion")
            sig = _lbl(nc.scalar.activation(rz[:, :], ps[:, 0:8 * bl], AF.Sigmoid),
                       f"sig l{l} i{i}")
            if i >= 3:
                _add_dep_helper(sig.ins, sa.ins, sync=False,
                                reason="WAW absorber precedes sigmoid")
            sig_hist[l][i] = sig

            # absorb the sigmoid tick into the DVE clock
            dabs = _lbl(dve_abs_read(rz[0:1, 0:1]), f"dve_abs_sig l{l} i{i}")
            pre_mul = [dabs]
            if i == 0:
                pre_mul.append(_lbl(
                    dve_abs_read(h0_sb[0:1, l * 4 * bl:l * 4 * bl + 1]),
                    f"dve_abs_h0 l{l}"))
            # n = tanh(gi_n + r * gh_n)
            nm = nmul_pools[l].tile([128, 4 * bl], F32, tag=f"nmul{l}")
            mul1 = _lbl(nc.vector.tensor_mul(nm[:, :], rz[:, 0:4 * bl],
                                             ps[:, 12 * bl:16 * bl]),
                        f"mul1 l{l} i{i}")
            for a in pre_mul:
                _add_dep_helper(mul1.ins, a.ins, sync=False,
                                reason="absorbers precede n-path mult")
            npre = npre_pools[l].tile([128, 4 * bl], F32, tag=f"npre{l}")
            add1 = _lbl(nc.vector.tensor_add(npre[:, :], nm[:, :],
                                             ps[:, 8 * bl:12 * bl]),
                        f"add1 l{l} i{i}")
            add1_hist[l][i] = add1
            ntl = nt_pools[l].tile([128, 4 * bl], BF16, tag=f"nt{l}")
            if i >= 3:
                ta = act_abs(tanh_hist[l][i - 3], "nt WAW completion")
            tanh = _lbl(nc.scalar.activation(ntl[:, :], npre[:, :], AF.Tanh),
                        f"tanh l{l} i{i}")
            if i >= 3:
                _add_dep_helper(tanh.ins, ta.ins, sync=False,
                                reason="WAW absorber precedes tanh")
            tanh_hist[l][i] = tanh
            last_eng['ACT'] = tanh

            # h' = n + z*(h - n)   (all bf16, SBUF)
            dt_ = d_pools[l].tile([128, 4 * bl], BF16, tag=f"d{l}")
            sub = _lbl(nc.vector.tensor_sub(dt_[:, :], ghsrc[:, :], ntl[:, :]),
                       f"sub l{l} i{i}")
            zd = zd_pools[l].tile([128, 4 * bl], BF16, tag=f"zd{l}")
            zdm = _lbl(nc.vector.tensor_mul(zd[:, :], rz[:, 4 * bl:8 * bl],
                                            dt_[:, :]), f"zd l{l} i{i}")
            pre_hp = []
            if l == 2 and i >= 8:
                # hT2 slot recycle: absorb the old out-DMA tick into DVE
                a = _lbl(dve_abs_read(nc.const_aps.scalar_like(0.0, dummy[0:1, 0:1])), f"dve_abs_odma i{i}")
                _add_dep_helper(a.ins, out_dmas[i - 8].ins, sync=True,
                                reason="hT2 WAR vs old out DMA")
                pre_hp.append(a)
            hnew = hT_pools[l].tile([128, 4 * bl], BF16, tag=f"hT{l}")
            hp = _lbl(nc.vector.tensor_add(hnew[:, :], zd[:, :], ntl[:, :]),
                      f"hp l{l} i{i}")
            for a in pre_hp:
                _add_dep_helper(hp.ins, a.ins, sync=False,
                                reason="out-DMA absorber precedes h'")
            last_eng['DVE'] = hp
            hT[l][i] = hnew
            hprod[l][i] = hp
            if i - 5 in hT[l] and l != 2:
                del hT[l][i - 5]

            if l == 2:
                prev_dma = None
                if len(out_dmas) >= 8:
                    c = dummy_ctr[0] % 2048
                    dummy_ctr[0] += 1
                    m = _lbl(nc.gpsimd.memset(dummy[0:1, c:c + 1], 0.0),
                             f"pool_abs i{i}")
                    _add_dep_helper(m.ins, out_dmas[-8].ins, sync=True,
                                    reason="SWDGE ring slot")
                    last_eng['POOL'] = m
                    prev_dma = m
                dma = _lbl(nc.gpsimd.dma_start(out_ext[i], hnew[:, :]),
                           f"out_dma i{i}")
                if prev_dma is not None:
                    _add_dep_helper(dma.ins, prev_dma.ins, sync=False,
                                    reason="ring absorber precedes out DMA")
                out_dmas.append(dma)
                if i - 9 in hT[2]:
                    del hT[2][i - 9]

        LAG = (0, 1, 3)
        for s in range(s_steps + LAG[2]):
            for l in range(3):
                i = s - LAG[l]
                if 0 <= i < s_steps:
                    emit_group(l, i)

        # ---- tail pre-drains (final Drain must see <=1 new wait) ---------
        tail = list(last_eng.values()) + out_dmas[-8:] + sp_dmas[-8:]
        for dep in tail:
            dr = nc.sync.drain(fusable=False)
            _add_dep_helper(dr.ins, dep.ins, sync=True, reason="tail pre-drain")

    return nc


# ---------------------------------------------------------------------------
# Host-side input preparation


def _fold_conv(conv_w, conv_b, w_ih0, b_ih0):
    """Fold conv into layer0 input weights: gi0[t] = W_eff @ x3[t] + b_eff."""
    RNN_IN = F * (NB - 2)
    C = np.zeros((RNN_IN, KX), np.float64)
    for f in range(F):
        for di in range(3):
            for dt in range(3):
                w = float(conv_w[f, 0, di, dt])
                for i in range(NB - 2):
                    C[f * (NB - 2) + i, dt * NB + (i + di)] += w
    W_eff = w_ih0.astype(np.float64) @ C  # [1536, 192]
    bc = np.repeat(conv_b.astype(np.float64), NB - 2)
    b_eff = b_ih0.astype(np.float64) + w_ih0.astype(np.float64) @ bc
    return W_eff.astype(np.float32), b_eff.astype(np.float32)


def _bf16(a):
    return np.ascontiguousarray(a.astype(ml_dtypes.bfloat16))


def _prep_core_inputs(inputs, s_steps=S, warm=WARM):
    x = np.asarray(inputs["x"], np.float32)
    W_eff, b_eff = _fold_conv(np.asarray(inputs["conv_w"], np.float32),
                              np.asarray(inputs["conv_b"], np.float32),
                              np.asarray(inputs["w_ih0"], np.float32),
                              np.asarray(inputs["b_ih0"], np.float32))

    def slab4(wT):
        # [512, 1536] -> [128, 4*1536] slab image (chunk-major cols)
        return wT.reshape(4, 128, G3).transpose(1, 0, 2).reshape(128, 4 * G3)

    wg0 = np.zeros((128, 2 * G3), np.float32)
    WeT = W_eff.T  # [192, 1536]
    wg0[:, 0:G3] = WeT[0:128]
    wg0[0:64, G3:2 * G3] = WeT[128:192]
    b_hh0 = np.asarray(inputs["b_hh0"], np.float32)
    wg0[64, G3:G3 + 1024] = (b_eff + b_hh0)[0:1024]
    wg0[64, G3 + 1024:2 * G3] = b_eff[1024:1536]
    wg12 = np.zeros((2, 128, 4 * G3), np.float32)
    for l in (1, 2):
        wg12[l - 1] = slab4(np.asarray(inputs[f"w_ih{l}"], np.float32).T)
    wh = np.zeros((3, 128, 4 * G3), np.float32)
    for l in range(3):
        wh[l] = slab4(np.asarray(inputs[f"w_hh{l}"], np.float32).T)

    bmat = np.zeros((16, 512), np.float32)
    for l in range(3):
        b_h = np.asarray(inputs[f"b_hh{l}"], np.float32)
        b_i = b_eff if l == 0 else np.asarray(inputs[f"b_ih{l}"], np.float32)
        col = slice(l * 128, (l + 1) * 128)
        both = b_i + b_h
        for k in range(8):
            bmat[k, col] = both[k * 128:(k + 1) * 128]
        for k in range(4):
            bmat[8 + k, col] = b_i[1024 + k * 128:1024 + (k + 1) * 128]
            bmat[12 + k, col] = b_h[1024 + k * 128:1024 + (k + 1) * 128]
    b_hh0 = np.asarray(inputs["b_hh0"], np.float32)
    for k in range(4):
        bmat[k, 384:512] = b_hh0[1024 + k * 128:1024 + (k + 1) * 128]
    sel = np.zeros((16, 640), np.float32)
    for k in range(16):
        sel[k, k * 32:(k + 1) * 32] = 1.0
    for k in range(4):
        sel[k, 512 + k * 32:512 + (k + 1) * 32] = 1.0

    wg0_b, wg12_b, wh_b = _bf16(wg0), _bf16(wg12), _bf16(wh)
    bmat_b, sel_b = _bf16(bmat), _bf16(sel)

    x2p = np.pad(x[:, 0], ((0, 0), (0, 0), (1, 1)))  # [B, NB, T+2]
    hs = [np.asarray(inputs[f"h{l + 1}"], np.float32) for l in range(3)]

    in_maps = []
    chunk_starts = [0] + [s_steps + (j - 1) * (s_steps - warm) - warm
                          for j in range(1, N_CHUNKS)]
    for bh in range(BATCH_WAYS):
        bsl = slice(bh * BL, (bh + 1) * BL)
        for j in range(N_CHUNKS):
            t0 = chunk_starts[j]
            seg = x2p[bsl, :, t0:t0 + s_steps + 2]  # [BL, 64, S+2]
            A = np.stack([seg[:, :, dt:dt + s_steps] for dt in range(3)], axis=0)
            w3T = A.transpose(0, 2, 1, 3).reshape(KX, BL, s_steps)  # [k, b, i]
            x3t = np.zeros((s_steps, 128, 64), np.float32)
            x3t[:, :, 0:32] = w3T[0:128].transpose(2, 0, 1)
            x3t[:, 0:64, 32:64] = w3T[128:192].transpose(2, 0, 1)
            x3t[:, 64, 32:64] = 1.0
            x3t = np.ascontiguousarray(
                x3t.transpose(1, 0, 2).reshape(128, s_steps * 64))
            h0t = np.zeros((3, 128, 4 * BL), np.float32)
            if j == 0:
                for l in range(3):
                    hT0 = hs[l][bsl].T  # [H, BL]
                    for c in range(4):
                        h0t[l, :, c * BL:(c + 1) * BL] = hT0[c * 128:(c + 1) * 128]
            in_maps.append({
                "wg0": wg0_b, "wg12": wg12_b, "wh": wh_b,
                "bmat": bmat_b, "sel": sel_b,
                "x3t": _bf16(x3t), "h0t": _bf16(h0t),
            })
    return in_maps, chunk_starts


def kernel(**inputs) -> np.ndarray:
    if "nc" not in _NC_CACHE:
        _NC_CACHE["nc"] = _build_nc()
    nc = _NC_CACHE["nc"]
    in_maps, chunk_starts = _prep_core_inputs(inputs)
    res = run_bass_kernel_spmd(nc, in_maps, list(range(8)))
    _NC_CACHE["last_result"] = res
    out = np.zeros((T, B, H), np.float32)
    for core, rmap in enumerate(res.results):
        bh, j = core // N_CHUNKS, core % N_CHUNKS
        bsl = slice(bh * BL, (bh + 1) * BL)
        o = np.asarray(rmap["out"])  # [S, 128, 4*BL] bf16
        # o[i, p, c*BL+b] = h3[b, c*128+p]
        o = np.asarray(o, dtype=np.float32).reshape(S, 128, 4, BL)
        o = o.transpose(0, 3, 2, 1).reshape(S, BL, H)
        if j == 0:
            out[0:S, bsl] = o
        else:
            lo = chunk_starts[j] + WARM
            out[lo:lo + (S - WARM), bsl] = o[WARM:]
    return out


# revision 30
# speedup vs baseline: 1.1496x; 1.0246x over previous
"""Trainium2 Bass kernel: Conv2d(1->64,3x3) + 3-layer GRU over T=256.

Weight-stationary formulation (v2):
  - Conv folded into layer-0 input weights host-side (gi0[t] = W_eff @ x3[t]).
  - 8 cores = 2 batch halves (32 each) x 4 time chunks with WARM-step warmup
    (GRU state contracts ~0.77/step, so warm chunks converge).
  - All GEMMs keep the WEIGHTS as the stationary operand and h as the moving
    operand, producing gates in [gate, batch] layout: each matmul streams only
    BL=32 columns, every elementwise op runs 128 partitions wide, and h never
    needs transposing (h' is produced directly in the layout the next step's
    matmuls consume).
  - Per (layer, step): one PSUM bank [128, 512] holds 16 blocks of 32 cols:
    blocks 0..7 = r|z pre-acts (gi+gh+bias), 8..11 = gi_n, 12..15 = gh_n.
    Biases enter via a single K=16 selector matmul that writes the whole bank
    (start=True), then gi/gh chunk matmuls accumulate block-wise.
  - Eltwise: sigmoid (ACT, psum->sbuf bf16), n = tanh(gi_n + r*gh_n) (DVE mul/
    add + ACT tanh), h' = n + z*(h-n) (DVE bf16). h state lives in bf16.
  - Wavefront: span s runs (l=0,t=s), (l=1,t=s-1), (l=2,t=s-2); all cross-
    group deps come from the previous span, so groups in a span are
    independent and the eltwise chain of one group hides under the PE stream
    of the others.
  - Single-sync-wait discipline (walrus limit): cheap absorber instructions
    (PE ldweights / 1-elem DVE copies / Pool memsets) carry all but one of
    each real instruction's cross-engine waits; Tile's wait assignment then
    elides the dominated ones.
  - Weights/bias/x3 stream in via SP-issued (HWDGE) DMAs; per-step output
    DMAs via gpsimd (SWDGE) read the layer-2 h tile directly.
"""

import sys

for _p in ("/opt/trn_rl_repo",):
    if _p not in sys.path:
        sys.path.insert(0, _p)

import numpy as np
import ml_dtypes

import concourse.bass as bass
import concourse.mybir as mybir
import concourse.tile as tile
from concourse.bass import _add_dep_helper
from concourse.bass_utils import run_bass_kernel_spmd

BF16 = mybir.dt.bfloat16
F32 = mybir.dt.float32
AF = mybir.ActivationFunctionType

B, NB, T, F, H = 64, 64, 256, 64, 512
G3 = 3 * H  # 1536
KX = 3 * NB  # 192 folded-conv contraction
BATCH_WAYS = 2
N_CHUNKS = 4
WARM = 24
S = (T + (N_CHUNKS - 1) * WARM) // N_CHUNKS  # 88 steps per core
BL = B // BATCH_WAYS  # 32 batch rows per core
NT = 12  # gate tiles of 128 (1536/128)

_NC_CACHE: dict = {}
_DBG_LABELS: dict = {}  # inst name -> semantic label (debug aid)


def _lbl(h, label):
    _DBG_LABELS[h.ins.name] = label
    return h


def _build_nc(s_steps: int = S, bl: int = BL):
    nc = bass.Bass()

    wg0_ext = nc.declare_dram_parameter("wg0", [128, 2 * G3], BF16, isOutput=False)
    wg12_ext = nc.declare_dram_parameter("wg12", [2, 128, 4 * G3], BF16,
                                         isOutput=False)
    wh_ext = nc.declare_dram_parameter("wh", [3, 128, 4 * G3], BF16, isOutput=False)
    bmat_ext = nc.declare_dram_parameter("bmat", [16, 512], BF16, isOutput=False)
    sel_ext = nc.declare_dram_parameter("sel", [16, 640], BF16, isOutput=False)
    x3t_ext = nc.declare_dram_parameter("x3t", [128, s_steps * 64], BF16,
                                       isOutput=False)
    h0t_ext = nc.declare_dram_parameter("h0t", [3, 128, 4 * bl], BF16, isOutput=False)
    out_ext = nc.declare_dram_parameter("out", [s_steps, 128, 4 * bl], BF16,
                                        isOutput=True)

    from contextlib import ExitStack

    with tile.TileContext(nc) as tc, ExitStack() as ctx:
        wpool = ctx.enter_context(tc.tile_pool(name="static", bufs=1))
        hT_pools = [
            ctx.enter_context(tc.tile_pool(name=f"hT{l}", bufs=(3, 6, 8)[l]))
            for l in range(3)
        ]
        rz_pools = [ctx.enter_context(tc.tile_pool(name=f"rz{l}", bufs=3))
                    for l in range(3)]
        nmul_pools = [ctx.enter_context(tc.tile_pool(name=f"nmul{l}", bufs=3))
                      for l in range(3)]
        npre_pools = [ctx.enter_context(tc.tile_pool(name=f"npre{l}", bufs=3))
                      for l in range(3)]
        nt_pools = [ctx.enter_context(tc.tile_pool(name=f"nt{l}", bufs=3))
                    for l in range(3)]
        d_pools = [ctx.enter_context(tc.tile_pool(name=f"d{l}", bufs=3))
                   for l in range(3)]
        zd_pools = [ctx.enter_context(tc.tile_pool(name=f"zd{l}", bufs=3))
                    for l in range(3)]
        ps_pools = [
            ctx.enter_context(tc.tile_pool(name=f"ps{l}", bufs=(4, 2, 2)[l], space="PSUM"))
            for l in range(3)
        ]

        # ---- static SBUF slabs ------------------------------------------
        wg0_sb = wpool.tile([128, 2 * G3], BF16, tag="wg0")
        wg12_sb = wpool.tile([128, 8 * G3], BF16, tag="wg12")
        wh_sb = wpool.tile([128, 12 * G3], BF16, tag="wh")
        bmat_sb = wpool.tile([128, 512], BF16, tag="bmat")
        sel_sb = wpool.tile([128, 640], BF16, tag="sel")
        x3_sb = wpool.tile([128, s_steps * 64], BF16, tag="x3")
        h0_sb = wpool.tile([128, 3 * 4 * bl], BF16, tag="h0")
        dummy = wpool.tile([1, 2048], F32, tag="dummy")
        act_dummy = wpool.tile([1, 2048], F32, tag="actdummy")
        dummy_ctr = [0]
        act_ctr = [0]

        # ---- preamble DMAs (SP / HWDGE) ---------------------------------
        sp_dmas = []

        def _sdma(dst, src):
            d = nc.sync.dma_start(dst, src)
            sp_dmas.append(d)
            return d

        wg_last = [None, None, None]  # last DMA per layer's gi slab
        wh_last = [None, None, None]
        x3_dma = [None] * s_steps

        bmat_dma = _sdma(bmat_sb[0:16, :], bmat_ext[:, :])
        sel_dma = _sdma(sel_sb[0:16, :], sel_ext[:, :])
        X3A = 8  # steps in the first (startup-critical) x3 image
        d = _sdma(x3_sb[:, 0:X3A * 64], x3t_ext[:, 0:X3A * 64])
        for i in range(X3A):
            x3_dma[i] = d
        h0_dma = [None] * 3
        for l in range(3):
            h0_dma[l] = _sdma(h0_sb[:, l * 4 * bl:(l + 1) * 4 * bl], h0t_ext[l])
        # chunk-granular weight DMAs so matmuls start as each chunk lands
        wg0_dmas = [_sdma(wg0_sb[:, c * G3:(c + 1) * G3],
                          wg0_ext[:, c * G3:(c + 1) * G3]) for c in range(2)]
        wg_last[0] = wg0_dmas[1]
        wh_dmas = [[None] * 4 for _ in range(3)]
        for c in range(4):
            wh_dmas[0][c] = _sdma(wh_sb[:, c * G3:(c + 1) * G3],
                                  wh_ext[0, :, c * G3:(c + 1) * G3])
        wg_last[1] = _sdma(wg12_sb[:, 0:4 * G3], wg12_ext[0])
        for c in range(4):
            wh_dmas[1][c] = _sdma(wh_sb[:, (4 + c) * G3:(5 + c) * G3],
                                  wh_ext[1, :, c * G3:(c + 1) * G3])
        wg_last[2] = _sdma(wg12_sb[:, 4 * G3:8 * G3], wg12_ext[1])
        for c in range(4):
            wh_dmas[2][c] = _sdma(wh_sb[:, (8 + c) * G3:(9 + c) * G3],
                                  wh_ext[2, :, c * G3:(c + 1) * G3])
        d = _sdma(x3_sb[:, X3A * 64:], x3t_ext[:, X3A * 64:])
        for i in range(X3A, s_steps):
            x3_dma[i] = d

        # PE absorbers so the first bias matmul never carries DMA waits
        for dep in (bmat_dma, sel_dma):
            a = nc.tensor.ldweights(bmat_sb[0:1, 0:1])
            _add_dep_helper(a.ins, dep.ins, sync=True, reason="preamble prime")

        # ---- bookkeeping -------------------------------------------------
        hT = [dict() for _ in range(3)]     # (l, i) -> h tile [128, 4*bl]
        hprod = [dict() for _ in range(3)]  # (l, i) -> producing instruction
        sig_hist = [dict() for _ in range(3)]
        tanh_hist = [dict() for _ in range(3)]
        add1_hist = [dict() for _ in range(3)]
        out_dmas = []
        last_eng = {}

        for l in range(3):
            hT[l][-1] = h0_sb[:, l * 4 * bl:(l + 1) * 4 * bl]

        def ldw_abs(dep, reason):
            a = nc.tensor.ldweights(bmat_sb[0:1, 0:1])
            _add_dep_helper(a.ins, dep.ins, sync=True, reason=reason)
            return a

        def dve_abs_read(src_ap):
            c = dummy_ctr[0] % 2048
            dummy_ctr[0] += 1
            return nc.vector.tensor_copy(dummy[0:1, c:c + 1], src_ap)

        act_zero = nc.const_aps.scalar_like(0.0, act_dummy[0:1, 0:1])

        def act_abs(dep, reason):
            c = act_ctr[0] % 2048
            act_ctr[0] += 1
            a = nc.scalar.activation(act_dummy[0:1, c:c + 1], act_zero, AF.Copy)
            _add_dep_helper(a.ins, dep.ins, sync=True, reason=reason)
            return a

        PSB = (4, 2, 2)  # psum bufs per layer

        def emit_group(l, i):
            # --- wait absorbers (keep every real instruction at <=1 wait) --
            # Only the psum-WAR and gi-input absorbers precede the bias/gi
            # matmuls; the recurrence absorber sits just before the gh
            # matmuls so h-independent gi work can run ahead of the chain.
            grp_abs = []
            if i == 0:
                grp_abs.append(ldw_abs(
                    wg0_dmas[0] if l == 0 else wg_last[l], f"wg{l} slab ready"))
            elif l >= 1:
                grp_abs.append(ldw_abs(hprod[l - 1][i], "h[l-1][i] ready"))
            if i >= PSB[l]:
                grp_abs.append(
                    ldw_abs(sig_hist[l][i - PSB[l]], "psum WAR vs old sig"))
                grp_abs.append(
                    ldw_abs(add1_hist[l][i - PSB[l]], "psum WAR vs old add1"))

            ps = ps_pools[l].tile([128, 512], F32, tag=f"ps{l}")

            # --- matmuls --------------------------------------------------
            # bias: psum[p, c] = bmat[c//32, p]. start=True marks the whole
            # bank pending-zero, so later start=False matmuls overwrite-on-
            # first-touch. l0 r|z|gin biases ride x3's ones row instead, so
            # its bias matmul only covers the gh_n blocks.
            if l == 0:
                mm_bias = _lbl(nc.tensor.matmul(
                    ps[:, 12 * bl:16 * bl], bmat_sb[0:4, 384:512],
                    sel_sb[0:4, 512:640], start=True, stop=False,
                    skip_group_check=True), f"mm_bias l{l} i{i}")
            else:
                mm_bias = _lbl(nc.tensor.matmul(
                    ps[:, :], bmat_sb[0:16, l * 128:(l + 1) * 128],
                    sel_sb[0:16, 0:512], start=True, stop=False,
                    skip_group_check=True), f"mm_bias l{l} i{i}")
            for a in grp_abs:
                _add_dep_helper(mm_bias.ins, a.ins, sync=False,
                                reason="group absorbers precede first matmul")

            if l == 0:
                gi_src = [(x3_sb[0:128, i * 64:i * 64 + 32], 0),
                          (x3_sb[0:65, i * 64 + 32:i * 64 + 64], 1)]

                def gi_w(c, j):
                    return wg0_sb[0:(128 if c == 0 else 65),
                                  c * G3 + j * 128:c * G3 + (j + 1) * 128]
            else:
                hsrc = hT[l - 1][i]
                gi_src = [(hsrc[:, c * bl:(c + 1) * bl], c) for c in range(4)]

                def gi_w(c, j):
                    base = ((l - 1) * 4 + c) * G3
                    return wg12_sb[:, base + j * 128:base + (j + 1) * 128]

            ghsrc = hT[l][i - 1]
            gh_src = [(ghsrc[:, c * bl:(c + 1) * bl], c) for c in range(4)]

            def gh_w(c, j):
                base = (l * 4 + c) * G3
                return wh_sb[:, base + j * 128:base + (j + 1) * 128]

            gi_mms = []  # h-independent for l=0; depends on h[l-1][i] else
            gh_mms = []  # recurrence-dependent
            if i == 0 and l == 0:
                for rhs, c in gi_src:
                    for j in range(0, 12):
                        gi_mms.append((gi_w(c, j), rhs, j))
            else:
                for j in range(0, 12):
                    for rhs, c in gi_src:
                        gi_mms.append((gi_w(c, j), rhs, j))
            # r|z gh blocks first (sigmoid can start mid-stream), gh_n last
            # (mul1's single PE wait lands on the final matmul); at i==0,
            # chunk-major so each wh chunk's matmuls start as its DMA lands
            if i == 0:
                for rhs, c in gh_src:
                    for j in range(0, 12):
                        gh_mms.append((gh_w(c, j), rhs, j + (4 if j >= 8 else 0)))
            else:
                for j in range(0, 8):
                    for rhs, c in gh_src:
                        gh_mms.append((gh_w(c, j), rhs, j))
                for j in range(8, 12):
                    for rhs, c in gh_src:
                        gh_mms.append((gh_w(c, j), rhs, j + 4))
            first_gi = None
            for idx, (w, rhs, blk) in enumerate(gi_mms):
                kk = w.shape[0]
                m = _lbl(nc.tensor.matmul(
                    ps[:, blk * bl:(blk + 1) * bl], w, rhs[0:kk, :],
                    start=False, stop=False, skip_group_check=True),
                    f"mm_gi l{l} i{i} #{idx} blk{blk}")
                if first_gi is None:
                    first_gi = m
                    for a in grp_abs:
                        _add_dep_helper(m.ins, a.ins, sync=False,
                                        reason="absorbers precede gi matmuls")
            # recurrence absorber gates only the gh matmuls
            if i == 0:
                rec_abs = [ldw_abs(h0_dma[l], "h0 slab ready")]
            else:
                rec_abs = [ldw_abs(hprod[l][i - 1], "h[l][i-1] ready")]
            n = len(gh_mms)
            prev_mm = None
            for idx, (w, rhs, blk) in enumerate(gh_mms):
                prev_mm = _lbl(nc.tensor.matmul(
                    ps[:, blk * bl:(blk + 1) * bl], w, rhs[:, :],
                    start=False, stop=(idx == n - 1), skip_group_check=True),
                    f"mm_gh l{l} i{i} #{idx} blk{blk}")
                if idx == 0:
                    for a in rec_abs:
                        _add_dep_helper(prev_mm.ins, a.ins, sync=False,
                                        reason="recurrence absorber precedes gh")
            last_eng['PE'] = prev_mm

            # --- eltwise --------------------------------------------------
            # r|z sigmoid -> sbuf bf16
            rz = rz_pools[l].tile([128, 2 * 4 * bl], BF16, tag=f"rz{l}")
            if i >= 3:
                sa = act_abs(sig_hist[l][i - 3], "rz WAW completion")
            sig = _lbl(nc.scalar.activation(rz[:, :], ps[:, 0:8 * bl], AF.Sigmoid),
                       f"sig l{l} i{i}")
            if i >= 3:
                _add_dep_helper(sig.ins, sa.ins, sync=False,
                                reason="WAW absorber precedes sigmoid")
            sig_hist[l][i] = sig

            # absorb the sigmoid tick into the DVE clock
            dabs = _lbl(dve_abs_read(rz[0:1, 0:1]), f"dve_abs_sig l{l} i{i}")
            pre_mul = [dabs]
            if i == 0:
                pre_mul.append(_lbl(
                    dve_abs_read(h0_sb[0:1, l * 4 * bl:l * 4 * bl + 1]),
                    f"dve_abs_h0 l{l}"))
            # n = tanh(gi_n + r * gh_n)
            nm = nmul_pools[l].tile([128, 4 * bl], F32, tag=f"nmul{l}")
            mul1 = _lbl(nc.vector.tensor_mul(nm[:, :], rz[:, 0:4 * bl],
                                             ps[:, 12 * bl:16 * bl]),
                        f"mul1 l{l} i{i}")
            for a in pre_mul:
                _add_dep_helper(mul1.ins, a.ins, sync=False,
                                reason="absorbers precede n-path mult")
            npre = npre_pools[l].tile([128, 4 * bl], F32, tag=f"npre{l}")
            add1 = _lbl(nc.vector.tensor_add(npre[:, :], nm[:, :],
                                             ps[:, 8 * bl:12 * bl]),
                        f"add1 l{l} i{i}")
            add1_hist[l][i] = add1
            ntl = nt_pools[l].tile([128, 4 * bl], BF16, tag=f"nt{l}")
            if i >= 3:
                ta = act_abs(tanh_hist[l][i - 3], "nt WAW completion")
            tanh = _lbl(nc.scalar.activation(ntl[:, :], npre[:, :], AF.Tanh),
                        f"tanh l{l} i{i}")
            if i >= 3:
                _add_dep_helper(tanh.ins, ta.ins, sync=False,
                                reason="WAW absorber precedes tanh")
            tanh_hist[l][i] = tanh
            last_eng['ACT'] = tanh

            # h' = n + z*(h - n)   (all bf16, SBUF)
            dt_ = d_pools[l].tile([128, 4 * bl], BF16, tag=f"d{l}")
            sub = _lbl(nc.vector.tensor_sub(dt_[:, :], ghsrc[:, :], ntl[:, :]),
                       f"sub l{l} i{i}")
            zd = zd_pools[l].tile([128, 4 * bl], BF16, tag=f"zd{l}")
            zdm = _lbl(nc.vector.tensor_mul(zd[:, :], rz[:, 4 * bl:8 * bl],
                                            dt_[:, :]), f"zd l{l} i{i}")
            pre_hp = []
            if l == 2 and i >= 8:
                # hT2 slot recycle: absorb the old out-DMA tick into DVE
                a = _lbl(dve_abs_read(nc.const_aps.scalar_like(0.0, dummy[0:1, 0:1])), f"dve_abs_odma i{i}")
                _add_dep_helper(a.ins, out_dmas[i - 8].ins, sync=True,
                                reason="hT2 WAR vs old out DMA")
                pre_hp.append(a)
            hnew = hT_pools[l].tile([128, 4 * bl], BF16, tag=f"hT{l}")
            hp = _lbl(nc.vector.tensor_add(hnew[:, :], zd[:, :], ntl[:, :]),
                      f"hp l{l} i{i}")
            for a in pre_hp:
                _add_dep_helper(hp.ins, a.ins, sync=False,
                                reason="out-DMA absorber precedes h'")
            last_eng['DVE'] = hp
            hT[l][i] = hnew
            hprod[l][i] = hp
            if i - 5 in hT[l] and l != 2:
                del hT[l][i - 5]

            if l == 2:
                prev_dma = None
                if len(out_dmas) >= 8:
                    c = dummy_ctr[0] % 2048
                    dummy_ctr[0] += 1
                    m = _lbl(nc.gpsimd.memset(dummy[0:1, c:c + 1], 0.0),
                             f"pool_abs i{i}")
                    _add_dep_helper(m.ins, out_dmas[-8].ins, sync=True,
                                    reason="SWDGE ring slot")
                    last_eng['POOL'] = m
                    prev_dma = m
                dma = _lbl(nc.gpsimd.dma_start(out_ext[i], hnew[:, :]),
                           f"out_dma i{i}")
                if prev_dma is not None:
                    _add_dep_helper(dma.ins, prev_dma.ins, sync=False,
                                    reason="ring absorber precedes out DMA")
                out_dmas.append(dma)
                if i - 9 in hT[2]:
                    del hT[2][i - 9]

        LAG = (0, 1, 3)
        for s in range(s_steps + LAG[2]):
            for l in range(3):
                i = s - LAG[l]
                if 0 <= i < s_steps:
                    emit_group(l, i)

        # ---- tail pre-drains (final Drain must see <=1 new wait) ---------
        tail = list(last_eng.values()) + out_dmas[-8:] + sp_dmas[-8:]
        for dep in tail:
            dr = nc.sync.drain(fusable=False)
            _add_dep_helper(dr.ins, dep.ins, sync=True, reason="tail pre-drain")

    return nc


# ---------------------------------------------------------------------------
# Host-side input preparation


def _fold_conv(conv_w, conv_b, w_ih0, b_ih0):
    """Fold conv into layer0 input weights: gi0[t] = W_eff @ x3[t] + b_eff."""
    RNN_IN = F * (NB - 2)
    C = np.zeros((RNN_IN, KX), np.float64)
    for f in range(F):
        for di in range(3):
            for dt in range(3):
                w = float(conv_w[f, 0, di, dt])
                for i in range(NB - 2):
                    C[f * (NB - 2) + i, dt * NB + (i + di)] += w
    W_eff = w_ih0.astype(np.float64) @ C  # [1536, 192]
    bc = np.repeat(conv_b.astype(np.float64), NB - 2)
    b_eff = b_ih0.astype(np.float64) + w_ih0.astype(np.float64) @ bc
    return W_eff.astype(np.float32), b_eff.astype(np.float32)


def _bf16(a):
    return np.ascontiguousarray(a.astype(ml_dtypes.bfloat16))


def _prep_core_inputs(inputs, s_steps=S, warm=WARM):
    x = np.asarray(inputs["x"], np.float32)
    W_eff, b_eff = _fold_conv(np.asarray(inputs["conv_w"], np.float32),
                              np.asarray(inputs["conv_b"], np.float32),
                              np.asarray(inputs["w_ih0"], np.float32),
                              np.asarray(inputs["b_ih0"], np.float32))

    def slab4(wT):
        # [512, 1536] -> [128, 4*1536] slab image (chunk-major cols)
        return wT.reshape(4, 128, G3).transpose(1, 0, 2).reshape(128, 4 * G3)

    wg0 = np.zeros((128, 2 * G3), np.float32)
    WeT = W_eff.T  # [192, 1536]
    wg0[:, 0:G3] = WeT[0:128]
    wg0[0:64, G3:2 * G3] = WeT[128:192]
    b_hh0 = np.asarray(inputs["b_hh0"], np.float32)
    wg0[64, G3:G3 + 1024] = (b_eff + b_hh0)[0:1024]
    wg0[64, G3 + 1024:2 * G3] = b_eff[1024:1536]
    wg12 = np.zeros((2, 128, 4 * G3), np.float32)
    for l in (1, 2):
        wg12[l - 1] = slab4(np.asarray(inputs[f"w_ih{l}"], np.float32).T)
    wh = np.zeros((3, 128, 4 * G3), np.float32)
    for l in range(3):
        wh[l] = slab4(np.asarray(inputs[f"w_hh{l}"], np.float32).T)

    bmat = np.zeros((16, 512), np.float32)
    for l in range(3):
        b_h = np.asarray(inputs[f"b_hh{l}"], np.float32)
        b_i = b_eff if l == 0 else np.asarray(inputs[f"b_ih{l}"], np.float32)
        col = slice(l * 128, (l + 1) * 128)
        both = b_i + b_h
        for k in range(8):
            bmat[k, col] = both[k * 128:(k + 1) * 128]
        for k in range(4):
            bmat[8 + k, col] = b_i[1024 + k * 128:1024 + (k + 1) * 128]
            bmat[12 + k, col] = b_h[1024 + k * 128:1024 + (k + 1) * 128]
    b_hh0 = np.asarray(inputs["b_hh0"], np.float32)
    for k in range(4):
        bmat[k, 384:512] = b_hh0[1024 + k * 128:1024 + (k + 1) * 128]
    sel = np.zeros((16, 640), np.float32)
    for k in range(16):
        sel[k, k * 32:(k + 1) * 32] = 1.0
    for k in range(4):
        sel[k, 512 + k * 32:512 + (k + 1) * 32] = 1.0

    wg0_b, wg12_b, wh_b = _bf16(wg0), _bf16(wg12), _bf16(wh)
    bmat_b, sel_b = _bf16(bmat), _bf16(sel)

    x2p = np.pad(x[:, 0], ((0, 0), (0, 0), (1, 1)))  # [B, NB, T+2]
    hs = [np.asarray(inputs[f"h{l + 1}"], np.float32) for l in range(3)]

    in_maps = []
    chunk_starts = [0] + [s_steps + (j - 1) * (s_steps - warm) - warm
                          for j in range(1, N_CHUNKS)]
    for bh in range(BATCH_WAYS):
        bsl = slice(bh * BL, (bh + 1) * BL)
        for j in range(N_CHUNKS):
            t0 = chunk_starts[j]
            seg = x2p[bsl, :, t0:t0 + s_steps + 2]  # [BL, 64, S+2]
            A = np.stack([seg[:, :, dt:dt + s_steps] for dt in range(3)], axis=0)
            w3T = A.transpose(0, 2, 1, 3).reshape(KX, BL, s_steps)  # [k, b, i]
            x3t = np.zeros((s_steps, 128, 64), np.float32)
            x3t[:, :, 0:32] = w3T[0:128].transpose(2, 0, 1)
            x3t[:, 0:64, 32:64] = w3T[128:192].transpose(2, 0, 1)
            x3t[:, 64, 32:64] = 1.0
            x3t = np.ascontiguousarray(
                x3t.transpose(1, 0, 2).reshape(128, s_steps * 64))
            h0t = np.zeros((3, 128, 4 * BL), np.float32)
            if j == 0:
                for l in range(3):
                    hT0 = hs[l][bsl].T  # [H, BL]
                    for c in range(4):
                        h0t[l, :, c * BL:(c + 1) * BL] = hT0[c * 128:(c + 1) * 128]
            in_maps.append({
                "wg0": wg0_b, "wg12": wg12_b, "wh": wh_b,
                "bmat": bmat_b, "sel": sel_b,
                "x3t": _bf16(x3t), "h0t": _bf16(h0t),
            })
    return in_maps, chunk_starts


def kernel(**inputs) -> np.ndarray:
    if "nc" not in _NC_CACHE:
        _NC_CACHE["nc"] = _build_nc()
    nc = _NC_CACHE["nc"]
    in_maps, chunk_starts = _prep_core_inputs(inputs)
    res = run_bass_kernel_spmd(nc, in_maps, list(range(8)))
    _NC_CACHE["last_result"] = res
    out = np.zeros((T, B, H), np.float32)
    for core, rmap in enumerate(res.results):
        bh, j = core // N_CHUNKS, core % N_CHUNKS
        bsl = slice(bh * BL, (bh + 1) * BL)
        o = np.asarray(rmap["out"])  # [S, 128, 4*BL] bf16
        # o[i, p, c*BL+b] = h3[b, c*128+p]
        o = np.asarray(o, dtype=np.float32).reshape(S, 128, 4, BL)
        o = o.transpose(0, 3, 2, 1).reshape(S, BL, H)
        if j == 0:
            out[0:S, bsl] = o
        else:
            lo = chunk_starts[j] + WARM
            out[lo:lo + (S - WARM), bsl] = o[WARM:]
    return out
